# revision 1
# baseline (speedup 1.0000x reference)
"""Trainium2 Bass kernel for nn_BiMP (GNN message passing), 8 NeuronCores SPMD.

Algorithm (validated against the reference by a numpy mirror):
  stage 1 (sparse TransformerConv on 4096 nodes, 131072 edges):
    - dst-sorted edges, sharded by dst range: core c owns dst nodes
      [512c, 512(c+1)), split in 4 windows of 128 nodes, each padded to a
      uniform per-window edge capacity (multiple of 128).
    - projections q|qWe|skip and k|v via float32r matmuls (x^T slices from
      host); kv table AllGathered to DRAM on every core.
    - per 128-edge tile: gather kv rows by src (multi-offset indirect DMA),
      expand q|qWe rows by dst via one-hot matmul (host-built S^T, bf16),
      alpha = rowsum(q*k)+ea*qWe (exp without max-subtract: alpha in [-9,9]),
      scatter-accumulate [ex | ex*ea | ex*v] with one-hot matmul (S, bf16)
      into a PSUM [128 nodes, 136]; denominator divided out after the sum.
  graph_norm1 computed redundantly per core on the AllGathered h^T.
  stage 2 (dense bipartite attention) in transposed layout [feat, nodes]:
    scoresT per (head, s-tile) via head-packed matmuls, exp on ACT (bf16),
    numerator/denominator matmuls, skip + graph_norm2 (stats AllReduce),
    adj = xt@xt.T per 256-row block (float32r), min/max AllReduce, normalize.

Self-contained: hardcodes all shapes; compiles the Bass program on first call
(cached per edge-capacity).
"""
import os
import sys
import types

import numpy as np


def _install_ntff_shim():
    """bass_utils imports antenv.axon_hooks when tracing; provide it."""
    if "antenv.axon_hooks" in sys.modules:
        return
    mod = types.ModuleType("antenv.axon_hooks")

    def set_axon_ntff_profile_hook(h):
        mod._hook = h

    def get_axon_ntff_profile_hook():
        return getattr(mod, "_hook", None)

    mod.set_axon_ntff_profile_hook = set_axon_ntff_profile_hook
    mod.get_axon_ntff_profile_hook = get_axon_ntff_profile_hook
    sys.modules["antenv.axon_hooks"] = mod
    try:
        import antenv
        antenv.axon_hooks = mod
        from trn_agent_boot.trn_boot import _ntff_profile_via_ctypes
        set_axon_ntff_profile_hook(_ntff_profile_via_ctypes("/opt/axon/libaxon_pjrt.so"))
    except Exception:
        pass


_install_ntff_shim()

import ml_dtypes
import concourse.bacc as bacc
import concourse.bass as bass
import concourse.mybir as mybir
import concourse.tile as tile
from concourse.bass_utils import run_bass_kernel_spmd
from concourse.masks import make_identity

dt = mybir.dt
bf16 = ml_dtypes.bfloat16

NS, NT, H, C = 4096, 2048, 4, 32
D = H * C            # 128
E1 = 131072
M = 8                # cores
NSL = NS // M        # 512 source nodes / core
NTL = NT // M        # 256 target rows / core
WIN = 128            # dst nodes per window
NWIN = NSL // WIN    # 4 windows / core
P = 128
ISQ = np.float32(1.0 / np.sqrt(np.float32(C)))
EPS_GN = np.float32(1e-5)

_prog_cache = {}


# --------------------------------------------------------------------------
# host-side preparation
# --------------------------------------------------------------------------

def _prep(inputs):
    x = np.ascontiguousarray(np.asarray(inputs["x"], np.float32))
    src = np.asarray(inputs["pos_edge_index"][0]).astype(np.int64)
    dst = np.asarray(inputs["pos_edge_index"][1]).astype(np.int64)
    ea = np.asarray(inputs["edge_attr"], np.float32).reshape(-1)
    xt_emb = np.asarray(inputs["target_node_embeddings"], np.float32)

    f32 = lambda k: np.asarray(inputs[k], np.float32)

    We = f32("e1_w").reshape(D)
    M2T = np.zeros((D, H), np.float32)
    for h in range(H):
        M2T[h * C:(h + 1) * C, h] = We[h * C:(h + 1) * C]

    Wq_s = f32("q1_w") * ISQ
    Wqe = Wq_s @ M2T
    W1 = np.ascontiguousarray(np.concatenate([Wq_s, Wqe, f32("skip1_w")], axis=1))  # [4096,260]
    W2 = np.ascontiguousarray(np.concatenate([f32("k1_w"), f32("v1_w")], axis=1))   # [4096,256]
    bq_s = f32("q1_b") * ISQ
    B1 = np.concatenate([bq_s, bq_s @ M2T, f32("skip1_b")]).reshape(1, 260)
    B2 = np.concatenate([f32("k1_b"), f32("v1_b")]).reshape(1, 256)

    # ---- edges: sort by dst, shard by window ----
    order = np.argsort(dst, kind="stable")
    src_s, dst_s, ea_s = src[order], dst[order], ea[order]
    win_id = dst_s // WIN
    counts = np.bincount(win_id, minlength=NS // WIN)
    cap = int(np.ceil(max(int(counts.max()), 128) / 128) * 128)
    ntile = cap // 128
    starts = np.zeros(NS // WIN + 1, np.int64)
    np.cumsum(counts, out=starts[1:])

    xT = x.T  # [feat, node] view

    per_core = []
    for c in range(M):
        eidx = np.zeros((P, NWIN * ntile), np.int32)
        ea_t = np.zeros((P, NWIN * ntile), np.float32)
        S_all = np.zeros((P, NWIN * ntile * P), bf16)
        ST_all = np.zeros((P, NWIN * ntile * P), bf16)
        for w in range(NWIN):
            g = c * NWIN + w
            lo, hi = starts[g], starts[g + 1]
            n = hi - lo
            s_pad = np.zeros(cap, np.int64)
            s_pad[:n] = src_s[lo:hi]
            d_pad = np.full(cap, -1, np.int64)
            d_pad[:n] = dst_s[lo:hi] - g * WIN
            e_pad = np.zeros(cap, np.float32)
            e_pad[:n] = ea_s[lo:hi]
            for j in range(ntile):
                sl = slice(j * P, (j + 1) * P)
                col = w * ntile + j
                eidx[:, col] = s_pad[sl]
                ea_t[:, col] = e_pad[sl]
                dj = d_pad[sl]
                valid = dj >= 0
                Sb = np.zeros((P, P), np.float32)
                Sb[np.arange(P)[valid], dj[valid]] = 1.0
                S_all[:, col * P:(col + 1) * P] = Sb.astype(bf16)
                ST_all[:, col * P:(col + 1) * P] = Sb.T.astype(bf16)
        m = {
            "xT_c": np.ascontiguousarray(xT[:, c * NSL:(c + 1) * NSL]),
            "W1": W1, "W2": W2, "B1": B1, "B2": B2,
            "S_all": S_all, "ST_all": ST_all,
            "eidx": eidx, "ea_t": ea_t,
            "xtT_c": np.ascontiguousarray(xt_emb[c * NTL:(c + 1) * NTL].T),
            "We_row": We.reshape(1, D),
            "q2w_s": np.ascontiguousarray(f32("q2_w") * ISQ),
            "sk2w": f32("skip2_w"),
            "sk2w_bf": f32("skip2_w").astype(bf16),
            "sk2b_bf": f32("skip2_b").reshape(1, D).astype(bf16),
            "k2w_bf": f32("k2_w").astype(bf16),
            "q2b_s": (f32("q2_b") * ISQ).reshape(1, D),
            "sk2b": f32("skip2_b").reshape(1, D),
            "k2b_bf": f32("k2_b").reshape(1, D).astype(bf16),
            "gn1_cols": np.stack([f32("gn1_w"), f32("gn1_b"), f32("gn1_ms")], axis=1),
            "ones_f": np.ones((1, 512), np.float32),
            "ones_bf": np.ones((1, 512), np.float32).astype(bf16),
        }
        # v2 augmented with a ones column per head: [v2_h | 1]
        v2wa = np.zeros((D, 4 * 33), np.float32)
        v2ba = np.zeros((1, 4 * 33), np.float32)
        v2w_np, v2b_np = f32("v2_w"), f32("v2_b")
        for h in range(H):
            v2wa[:, 33 * h:33 * h + 32] = v2w_np[:, 32 * h:32 * (h + 1)]
            v2ba[0, 33 * h:33 * h + 32] = v2b_np[32 * h:32 * (h + 1)]
            v2ba[0, 33 * h + 32] = 1.0
        m["v2w_aug"] = v2wa.astype(bf16)
        m["v2b_aug"] = v2ba.astype(bf16)
        gn2_hs = np.zeros((32, 3 * H), np.float32)
        for h in range(H):
            gn2_hs[:, 3 * h + 0] = f32("gn2_w")[32 * h:32 * (h + 1)]
            gn2_hs[:, 3 * h + 1] = f32("gn2_b")[32 * h:32 * (h + 1)]
            gn2_hs[:, 3 * h + 2] = f32("gn2_ms")[32 * h:32 * (h + 1)]
        m["gn2_hs"] = gn2_hs
        per_core.append(m)
    return per_core, ntile


# --------------------------------------------------------------------------
# program builder
# --------------------------------------------------------------------------


def _build(ntile, debug=False, stop=None):
    nc = bacc.Bacc("TRN2", target_bir_lowering=False, debug=False, num_devices=M)
    fr = dt.float32r
    f32 = dt.float32
    b16 = dt.bfloat16
    SA = {"p1": 1, "edge": 2, "gn1": 3, "proj2": 4, "attn": 5}.get(stop, 6)

    # ---- I/O ----
    xT_c = nc.dram_tensor("xT_c", [NS, NSL], fr, kind="ExternalInput")
    W1 = nc.dram_tensor("W1", [NS, 260], fr, kind="ExternalInput")
    W2 = nc.dram_tensor("W2", [NS, 256], fr, kind="ExternalInput")
    B1 = nc.dram_tensor("B1", [1, 260], fr, kind="ExternalInput")
    B2 = nc.dram_tensor("B2", [1, 256], fr, kind="ExternalInput")
    S_all = nc.dram_tensor("S_all", [P, NWIN * ntile * P], b16, kind="ExternalInput")
    ST_all = nc.dram_tensor("ST_all", [P, NWIN * ntile * P], b16, kind="ExternalInput")
    eidx = nc.dram_tensor("eidx", [P, NWIN * ntile], dt.int32, kind="ExternalInput")
    ea_t = nc.dram_tensor("ea_t", [P, NWIN * ntile], f32, kind="ExternalInput")
    xtT_c = nc.dram_tensor("xtT_c", [D, NTL], fr, kind="ExternalInput")
    We_row = nc.dram_tensor("We_row", [1, D], f32, kind="ExternalInput")
    q2w_s = nc.dram_tensor("q2w_s", [D, D], fr, kind="ExternalInput")
    sk2w = nc.dram_tensor("sk2w", [D, D], fr, kind="ExternalInput")
    sk2w_b16 = nc.dram_tensor("sk2w_bf", [D, D], b16, kind="ExternalInput")
    sk2b_b16 = nc.dram_tensor("sk2b_bf", [1, D], b16, kind="ExternalInput")
    k2w_bf = nc.dram_tensor("k2w_bf", [D, D], b16, kind="ExternalInput")
    v2w_aug = nc.dram_tensor("v2w_aug", [D, 4 * 33], b16, kind="ExternalInput")
    v2b_aug = nc.dram_tensor("v2b_aug", [1, 4 * 33], b16, kind="ExternalInput")
    q2b_s = nc.dram_tensor("q2b_s", [1, D], fr, kind="ExternalInput")
    sk2b = nc.dram_tensor("sk2b", [1, D], fr, kind="ExternalInput")
    k2b_bf = nc.dram_tensor("k2b_bf", [1, D], b16, kind="ExternalInput")
    gn1_cols = nc.dram_tensor("gn1_cols", [D, 3], f32, kind="ExternalInput")
    gn2_hs_d = nc.dram_tensor("gn2_hs", [32, 3 * H], f32, kind="ExternalInput")
    ones_f = nc.dram_tensor("ones_f", [1, 512], fr, kind="ExternalInput")
    ones_bf = nc.dram_tensor("ones_bf", [1, 512], b16, kind="ExternalInput")

    adj_out = nc.dram_tensor("adj_out", [NTL, NT], f32, kind="ExternalOutput")
    if debug:
        dbg_kv = nc.dram_tensor("dbg_kv", [NSL, 256], f32, kind="ExternalOutput")
        dbg_hT = nc.dram_tensor("dbg_hT", [P, NSL], f32, kind="ExternalOutput")
        dbg_xtp = nc.dram_tensor("dbg_xtp", [P, NTL], f32, kind="ExternalOutput")

    # internal DRAM (collective bounce buffers)
    kv_loc = nc.dram_tensor("kv_loc", [NSL, 256], f32)
    kv_full = nc.dram_tensor("kv_full", [NS, 256], f32, addr_space="Shared")
    hT_loc = nc.dram_tensor("hT_loc", [P, NSL], f32)
    hT_stack = nc.dram_tensor("hT_stack", [M * P, NSL], f32, addr_space="Shared")
    st_loc = nc.dram_tensor("st_loc", [32, 8], f32)
    st_full = nc.dram_tensor("st_full", [32, 8], f32, addr_space="Shared")
    xtT_loc = nc.dram_tensor("xtT_loc", [32, H * NTL], b16)
    xtT_stack = nc.dram_tensor("xtT_stack", [M * 32, H * NTL], b16, addr_space="Shared")
    mm_loc = nc.dram_tensor("mm_loc", [1, 8], f32)
    mm_full = nc.dram_tensor("mm_full", [1, 8], f32, addr_space="Shared")

    rg = [list(range(M))]

    with tile.TileContext(nc) as tc:
        with (
            tc.tile_pool(name="persist", bufs=1) as pp,
            tc.tile_pool(name="ps_persist", bufs=1, space="PSUM") as pps,
        ):
            # persistent small tiles
            qq_bf = pp.tile([P, NWIN * 132], b16, tag="qq")
            skip_sb = pp.tile([P, NWIN * D], f32, tag="skip1")
            ea_sb = pp.tile([P, NWIN * ntile], f32, tag="ea")
            nc.sync.dma_start(ea_sb[:], ea_t[:])
            eidx_sb = pp.tile([P, NWIN * ntile], dt.int32, tag="eidx")
            nc.sync.dma_start(eidx_sb[:], eidx[:])
            ones_f_sb = pp.tile([1, 512], fr, tag="ones_f")
            nc.sync.dma_start(ones_f_sb[:], ones_f[:])
            ones_bf_sb = pp.tile([1, 512], b16, tag="ones_bf")
            nc.sync.dma_start(ones_bf_sb[:], ones_bf[:])
            We_sb = pp.tile([1, D], f32, tag="We_row")
            nc.sync.dma_start(We_sb[:], We_row[:])
            gn1_sb = pp.tile([D, 3], f32, tag="gn1")
            nc.sync.dma_start(gn1_sb[:], gn1_cols[:])
            gn2_hs_sb = pp.tile([32, 3 * H], f32, tag="gn2hs")
            nc.sync.dma_start(gn2_hs_sb[:], gn2_hs_d[:])
            ident = pp.tile([P, P], f32, tag="ident")
            make_identity(nc, ident)
            hT_local = pp.tile([P, NSL], f32, tag="hT_local")
            ones_f32_row = pp.tile([1, P], f32, tag="ones_f32r")
            nc.vector.memset(ones_f32_row[:], 1.0)
            We_rep = pp.tile([P, P], f32, tag="We_rep")

            # ============ P1: projections ============
            with (
                tc.tile_pool(name="wpool", bufs=1) as wp,
                tc.tile_pool(name="p1sb", bufs=4) as p1,
                tc.tile_pool(name="p1ps", bufs=2, space="PSUM") as p1ps,
            ):
                W1_sb = wp.tile([P, 32 * 260], fr, tag="W1")
                W2_sb = wp.tile([P, 32 * 256], fr, tag="W2")
                for kt in range(32):
                    nc.sync.dma_start(W1_sb[:, kt * 260:(kt + 1) * 260],
                                      W1[kt * P:(kt + 1) * P, :])
                    nc.sync.dma_start(W2_sb[:, kt * 256:(kt + 1) * 256],
                                      W2[kt * P:(kt + 1) * P, :])
                B1_sb = wp.tile([1, 260], fr, tag="B1")
                nc.sync.dma_start(B1_sb[:], B1[:])
                B2_sb = wp.tile([1, 256], fr, tag="B2")
                nc.sync.dma_start(B2_sb[:], B2[:])

                # We replicated [128,128]
                werep_ps = p1ps.tile([P, 260], f32, space="PSUM", tag="ps1")
                nc.tensor.matmul(werep_ps[:, :P], ones_f32_row[:], We_sb[:],
                                 start=True, stop=True)
                nc.vector.tensor_copy(We_rep[:], werep_ps[:, :P])

                for mt in range(NWIN):
                    ps1 = p1ps.tile([P, 260], f32, space="PSUM", tag="ps1")
                    ps2 = p1ps.tile([P, 256], f32, space="PSUM", tag="ps2")
                    nc.tensor.matmul(ps1[:], ones_f_sb[:, :P], B1_sb[:],
                                     start=True, stop=False)
                    nc.tensor.matmul(ps2[:], ones_f_sb[:, :P], B2_sb[:],
                                     start=True, stop=False)
                    for kt in range(32):
                        xt_t = p1.tile([P, P], fr, tag="xt")
                        nc.sync.dma_start(
                            xt_t[:], xT_c[kt * P:(kt + 1) * P, mt * P:(mt + 1) * P])
                        nc.tensor.matmul(ps1[:], xt_t[:],
                                         W1_sb[:, kt * 260:(kt + 1) * 260],
                                         start=False, stop=(kt == 31))
                        nc.tensor.matmul(ps2[:], xt_t[:],
                                         W2_sb[:, kt * 256:(kt + 1) * 256],
                                         start=False, stop=(kt == 31))
                    nc.vector.tensor_copy(qq_bf[:, mt * 132:(mt + 1) * 132],
                                          ps1[:, 0:132])
                    nc.vector.tensor_copy(skip_sb[:, mt * D:(mt + 1) * D],
                                          ps1[:, 132:260])
                    kv_st = p1.tile([P, 256], f32, tag="kvst")
                    nc.vector.tensor_copy(kv_st[:], ps2[:])
                    nc.sync.dma_start(kv_loc[mt * P:(mt + 1) * P, :], kv_st[:])

            # ============ P2: AllGather kv ============
            nc.gpsimd.collective_compute(
                "AllGather", mybir.AluOpType.bypass,
                ins=[kv_loc[:]], outs=[kv_full[:]], replica_groups=rg)
            if debug:
                nc.sync.dma_start(dbg_kv[:], kv_loc[:])

            # ============ P3: edge phase ============
            if SA >= 2:
                with (
                    tc.tile_pool(name="edgesb", bufs=2) as ep,
                    tc.tile_pool(name="edgesm", bufs=3) as esm,
                    tc.tile_pool(name="edgeps", bufs=2, space="PSUM") as eps,
                    tc.tile_pool(name="aggps", bufs=2, space="PSUM") as aps,
                ):
                    for w in range(NWIN):
                        gkv = ep.tile([P, ntile * 256], f32, tag="gkv")
                        nc.gpsimd.indirect_dma_start(
                            out=gkv[:], out_offset=None, in_=kv_full[:],
                            in_offset=bass.IndirectOffsetOnAxis(
                                ap=eidx_sb[:, w * ntile:(w + 1) * ntile], axis=0))
                        S_sb = ep.tile([P, ntile * P], b16, tag="S")
                        nc.sync.dma_start(
                            S_sb[:], S_all[:, w * ntile * P:(w + 1) * ntile * P])
                        ST_sb = ep.tile([P, ntile * P], b16, tag="ST")
                        nc.sync.dma_start(
                            ST_sb[:], ST_all[:, w * ntile * P:(w + 1) * ntile * P])

                        agg_ps = aps.tile([P, 136], f32, space="PSUM", tag="agg")
                        for j in range(ntile):
                            qexp = eps.tile([P, 132], f32, space="PSUM", tag="qexp")
                            nc.tensor.matmul(
                                qexp[:], ST_sb[:, j * P:(j + 1) * P],
                                qq_bf[:, w * 132:(w + 1) * 132],
                                start=True, stop=True)
                            ea_col = ea_sb[:, w * ntile + j:w * ntile + j + 1]
                            qk = esm.tile([P, D], f32, tag="qk")
                            nc.vector.tensor_tensor(
                                out=qk[:], in0=qexp[:, 0:D],
                                in1=gkv[:, j * 256:j * 256 + D],
                                op=mybir.AluOpType.mult)
                            al4 = esm.tile([P, 2 * H], f32, tag="al4")
                            nc.vector.reduce_sum(
                                out=al4[:, 0:H],
                                in_=qk[:].rearrange("p (h c) -> p h c", h=H),
                                axis=mybir.AxisListType.X)
                            nc.vector.tensor_scalar_mul(
                                al4[:, H:2 * H], qexp[:, D:D + H], ea_col)
                            nc.vector.tensor_tensor(
                                out=al4[:, 0:H], in0=al4[:, 0:H], in1=al4[:, H:2 * H],
                                op=mybir.AluOpType.add)
                            rhs = esm.tile([P, 136], b16, tag="rhs")
                            nc.scalar.activation(
                                rhs[:, 0:H], al4[:, 0:H],
                                mybir.ActivationFunctionType.Exp)
                            nc.vector.tensor_scalar_mul(
                                rhs[:, H:2 * H], rhs[:, 0:H], ea_col)
                            nc.vector.tensor_tensor(
                                out=rhs[:, 8:136], in0=gkv[:, j * 256 + D:(j + 1) * 256],
                                in1=rhs[:, 0:H].unsqueeze(2).to_broadcast([P, H, C]),
                                op=mybir.AluOpType.mult)
                            nc.tensor.matmul(
                                agg_ps[:], S_sb[:, j * P:(j + 1) * P], rhs[:],
                                start=(j == 0), stop=(j == ntile - 1))

                        # finalize window
                        invd = esm.tile([P, H], f32, tag="invd")
                        nc.vector.reciprocal(invd[:], agg_ps[:, 0:H])
                        s2we = esm.tile([P, D], f32, tag="s2we")
                        nc.vector.tensor_tensor(
                            out=s2we[:],
                            in0=agg_ps[:, H:2 * H].unsqueeze(2).to_broadcast([P, H, C]),
                            in1=We_rep[:], op=mybir.AluOpType.mult)
                        hpre = esm.tile([P, D], f32, tag="hpre")
                        nc.vector.tensor_tensor(
                            out=hpre[:], in0=agg_ps[:, 8:136], in1=s2we[:],
                            op=mybir.AluOpType.add)
                        nc.vector.tensor_tensor(
                            out=hpre[:], in0=hpre[:],
                            in1=invd[:].unsqueeze(2).to_broadcast([P, H, C]),
                            op=mybir.AluOpType.mult)
                        nc.vector.tensor_tensor(
                            out=hpre[:], in0=hpre[:],
                            in1=skip_sb[:, w * D:(w + 1) * D],
                            op=mybir.AluOpType.add)
                        tr_ps = eps.tile([P, P], f32, space="PSUM", tag="tr")
                        nc.tensor.transpose(tr_ps[:], hpre[:], ident[:])
                        nc.vector.tensor_copy(hT_local[:, w * P:(w + 1) * P], tr_ps[:])

            if SA >= 3:
                # ============ P4: AllGather hT + gn1 ============
                nc.sync.dma_start(hT_loc[:], hT_local[:])
                nc.gpsimd.collective_compute(
                    "AllGather", mybir.AluOpType.bypass,
                    ins=[hT_loc[:]], outs=[hT_stack[:]], replica_groups=rg)
            if debug and SA >= 3:
                nc.sync.dma_start(dbg_hT[:], hT_local[:])

            with (
                tc.tile_pool(name="s2sb", bufs=1) as s2,
                tc.tile_pool(name="s2sm", bufs=2) as s2m,
                tc.tile_pool(name="s2ps", bufs=2, space="PSUM") as s2ps,
                tc.tile_pool(name="scpool", bufs=1, space="PSUM") as scpool,
                tc.tile_pool(name="ndpool", bufs=1, space="PSUM") as ndpool,
            ):
              if SA >= 3:
                hT_full = s2.tile([P, NS], f32, tag="hT_full")
                for r in range(M):
                    nc.sync.dma_start(hT_full[:, r * NSL:(r + 1) * NSL],
                                      hT_stack[r * P:(r + 1) * P, :])
                # gn1 stats (fp32, redundant per core)
                mean = s2m.tile([P, 1], f32, tag="gnm")
                nc.vector.reduce_sum(out=mean[:], in_=hT_full[:],
                                     axis=mybir.AxisListType.X)
                nc.vector.tensor_scalar_mul(mean[:], mean[:], float(1.0 / NS))
                msmean = s2m.tile([P, 1], f32, tag="gnmm")
                nc.vector.tensor_tensor(out=msmean[:], in0=mean[:],
                                        in1=gn1_sb[:, 2:3],
                                        op=mybir.AluOpType.mult)
                xc = hT_full
                nc.vector.tensor_scalar_sub(xc[:], hT_full[:], msmean[:, 0:1])
                sq_scr = s2.tile([P, NS], b16, tag="sqscr")
                sumsq = s2m.tile([P, 1], f32, tag="sumsq")
                nc.scalar.activation(
                    sq_scr[:], xc[:], mybir.ActivationFunctionType.Square,
                    accum_out=sumsq[:])
                var = s2m.tile([P, 1], f32, tag="var")
                nc.vector.tensor_scalar(
                    out=var[:], in0=sumsq[:], scalar1=float(1.0 / NS),
                    scalar2=float(EPS_GN), op0=mybir.AluOpType.mult,
                    op1=mybir.AluOpType.add)
                std = s2m.tile([P, 1], f32, tag="std")
                nc.scalar.sqrt(std[:], var[:])
                rstd = s2m.tile([P, 1], f32, tag="rstd")
                nc.vector.reciprocal(rstd[:], std[:])
                scale1 = s2m.tile([P, 1], f32, tag="scale1")
                nc.vector.tensor_tensor(out=scale1[:], in0=gn1_sb[:, 0:1],
                                        in1=rstd[:], op=mybir.AluOpType.mult)
                hT_bf = s2.tile([P, NS], b16, tag="hT_bf")
                nc.scalar.activation(hT_bf[:], xc[:],
                                     mybir.ActivationFunctionType.Relu,
                                     bias=gn1_sb[:, 1:2], scale=scale1[:, 0:1])
                xcl = s2m.tile([P, NSL], f32, tag="xcl")
                nc.vector.tensor_scalar_sub(xcl[:], hT_local[:], msmean[:, 0:1])
                hT_bf_loc = s2m.tile([P, NSL], b16, tag="hT_bf_loc")
                nc.scalar.activation(hT_bf_loc[:], xcl[:],
                                     mybir.ActivationFunctionType.Relu,
                                     bias=gn1_sb[:, 1:2], scale=scale1[:, 0:1])

              if SA >= 4:
                # ---- stage-2 projections (head-split, all base-partition 0) ----
                k2w_sb = s2m.tile([D, D], b16, tag="k2w")
                nc.sync.dma_start(k2w_sb[:], k2w_bf[:])
                v2wa_sb = s2m.tile([D, 132], b16, tag="v2wa")
                nc.sync.dma_start(v2wa_sb[:], v2w_aug[:])
                v2ba_sb = s2m.tile([1, 132], b16, tag="v2ba")
                nc.sync.dma_start(v2ba_sb[:], v2b_aug[:])
                q2w_sb = s2m.tile([D, D], fr, tag="q2w")
                nc.sync.dma_start(q2w_sb[:], q2w_s[:])
                sk2w_sb = s2m.tile([D, D], fr, tag="sk2w")
                nc.sync.dma_start(sk2w_sb[:], sk2w[:])
                k2b_sb = s2m.tile([1, D], b16, tag="k2b")
                nc.sync.dma_start(k2b_sb[:], k2b_bf[:])
                q2b_sb = s2m.tile([1, D], fr, tag="q2b")
                nc.sync.dma_start(q2b_sb[:], q2b_s[:])
                sk2b_sb = s2m.tile([1, D], fr, tag="sk2b")
                nc.sync.dma_start(sk2b_sb[:], sk2b[:])
                sk2w_bf_sb = s2m.tile([D, D], b16, tag="sk2w_bf")
                nc.sync.dma_start(sk2w_bf_sb[:], sk2w_b16[:])
                sk2b_bf_sb = s2m.tile([1, D], b16, tag="sk2b_bf")
                nc.sync.dma_start(sk2b_bf_sb[:], sk2b_b16[:])
                xtT_sb = s2m.tile([D, NTL], fr, tag="xtT")
                nc.sync.dma_start(xtT_sb[:], xtT_c[:])

                # k2T head-split: [32, 4*4096] bf16
                k2T_hs = s2.tile([32, H * NS], b16, tag="k2T")
                for h in range(H):
                    for ch in range(8):
                        kps = s2ps.tile([P, 512], f32, space="PSUM", tag="ps512")
                        nc.tensor.matmul(kps[:32, :], k2b_sb[:, 32 * h:32 * (h + 1)],
                                         ones_bf_sb[:], start=True, stop=False)
                        nc.tensor.matmul(kps[:32, :],
                                         k2w_sb[:, 32 * h:32 * (h + 1)],
                                         hT_bf[:, ch * 512:(ch + 1) * 512],
                                         start=False, stop=True)
                        nc.vector.tensor_copy(
                            k2T_hs[:, h * NS + ch * 512:h * NS + (ch + 1) * 512],
                            kps[:32, :])
                # q2T head-split: [32, 4*256] bf16
                q2T_hs = s2m.tile([32, H * NTL], b16, tag="q2T")
                for h in range(H):
                    qps = s2ps.tile([P, 512], f32, space="PSUM", tag="ps512")
                    nc.tensor.matmul(qps[:32, :NTL],
                                     q2b_sb[:, 32 * h:32 * (h + 1)],
                                     ones_f_sb[:, :NTL], start=True, stop=False)
                    nc.tensor.matmul(qps[:32, :NTL],
                                     q2w_sb[:, 32 * h:32 * (h + 1)], xtT_sb[:],
                                     start=False, stop=True)
                    nc.vector.tensor_copy(q2T_hs[:, h * NTL:(h + 1) * NTL],
                                          qps[:32, :NTL])
                # v2 augmented [s, 4*(32+1)] bf16 per s-tile
                v2a_sb = s2.tile([P, 32 * 132], b16, tag="v2a")
                for st in range(32):
                    vps = s2ps.tile([P, 512], f32, space="PSUM", tag="ps512")
                    nc.tensor.matmul(vps[:, :132], ones_bf_sb[:, :P], v2ba_sb[:],
                                     start=True, stop=False)
                    nc.tensor.matmul(vps[:, :132], hT_bf[:, st * P:(st + 1) * P],
                                     v2wa_sb[:], start=False, stop=True)
                    nc.vector.tensor_copy(v2a_sb[:, st * 132:(st + 1) * 132],
                                          vps[:, :132])
                # target skip head-split -> xtpT_hs [32, 4*256] f32
                xtpT_hs = s2.tile([32, H * NTL], f32, tag="xtpT")
                for h in range(H):
                    sps = s2ps.tile([P, 512], f32, space="PSUM", tag="ps512")
                    nc.tensor.matmul(sps[:32, :NTL],
                                     sk2b_sb[:, 32 * h:32 * (h + 1)],
                                     ones_f_sb[:, :NTL], start=True, stop=False)
                    nc.tensor.matmul(sps[:32, :NTL],
                                     sk2w_sb[:, 32 * h:32 * (h + 1)], xtT_sb[:],
                                     start=False, stop=True)
                    nc.vector.tensor_copy(xtpT_hs[:, h * NTL:(h + 1) * NTL],
                                          sps[:32, :NTL])

              if SA >= 5:
                # ---- attention: per-head scoresT / exp / fused numer+den ----
                nd_ps = [ndpool.tile([33, NTL], f32, space="PSUM", tag=f"nd{h}",
                                     name=f"nd_ps{h}")
                         for h in range(H)]
                for st in range(32):
                    scps = scpool.tile([P, H * NTL], f32, space="PSUM", tag="sc1024")
                    for h in range(H):
                        nc.tensor.matmul(
                            scps[:, h * NTL:(h + 1) * NTL],
                            k2T_hs[:, h * NS + st * P:h * NS + (st + 1) * P],
                            q2T_hs[:, h * NTL:(h + 1) * NTL],
                            start=True, stop=True)
                    exp_sb = s2m.tile([P, H * NTL], b16, tag="expT")
                    nc.scalar.activation(exp_sb[:], scps[:],
                                         mybir.ActivationFunctionType.Exp)
                    for h in range(H):
                        nc.tensor.matmul(
                            nd_ps[h][:],
                            v2a_sb[:, st * 132 + 33 * h:st * 132 + 33 * (h + 1)],
                            exp_sb[:, h * NTL:(h + 1) * NTL],
                            start=(st == 0), stop=(st == 31))

                # per head: replicate denominator, divide, accumulate into xtpT
                for h in range(H):
                    denrow = s2m.tile([1, NTL], f32, tag="denrow")
                    nc.vector.tensor_copy(denrow[:], nd_ps[h][32:33, :])
                    drep_ps = s2ps.tile([P, 512], f32, space="PSUM", tag="ps512")
                    nc.tensor.matmul(drep_ps[:, :NTL], ones_f32_row[:], denrow[:],
                                     start=True, stop=True)
                    invd = s2m.tile([32, NTL], f32, tag="invdh")
                    nc.vector.reciprocal(invd[:], drep_ps[:32, :NTL])
                    oth = s2m.tile([32, NTL], f32, tag="oth")
                    nc.vector.tensor_tensor(out=oth[:], in0=nd_ps[h][0:32, :],
                                            in1=invd[:], op=mybir.AluOpType.mult)
                    nc.vector.tensor_tensor(
                        out=xtpT_hs[:, h * NTL:(h + 1) * NTL],
                        in0=xtpT_hs[:, h * NTL:(h + 1) * NTL],
                        in1=oth[:], op=mybir.AluOpType.add)
                if debug:
                    nc.sync.dma_start(dbg_xtp[:32, :], xtpT_hs[:, 0:NTL])

              if SA >= 6:
                # ---- gn2 stats (head-split [32, 8]: per head [sum, sumsq]) ----
                ssum = s2m.tile([32, 8], f32, tag="ssum")
                nc.vector.memset(ssum[:], 0.0)
                scr512 = s2m.tile([32, NSL], b16, tag="scr512")
                part = s2m.tile([32, 1], f32, tag="part")
                for h in range(H):
                    skq = s2ps.tile([P, 512], f32, space="PSUM", tag="ps512")
                    nc.tensor.matmul(skq[:32, :],
                                     sk2b_bf_sb[:, 32 * h:32 * (h + 1)],
                                     ones_bf_sb[:], start=True, stop=False)
                    nc.tensor.matmul(skq[:32, :],
                                     sk2w_bf_sb[:, 32 * h:32 * (h + 1)],
                                     hT_bf_loc[:], start=False, stop=True)
                    nc.vector.reduce_sum(out=part[:], in_=skq[:32, :],
                                         axis=mybir.AxisListType.X)
                    nc.vector.tensor_tensor(
                        out=ssum[:, 2 * h:2 * h + 1], in0=ssum[:, 2 * h:2 * h + 1],
                        in1=part[:], op=mybir.AluOpType.add)
                    nc.scalar.activation(
                        scr512[:], skq[:32, :],
                        mybir.ActivationFunctionType.Square, accum_out=part[:])
                    nc.vector.tensor_tensor(
                        out=ssum[:, 2 * h + 1:2 * h + 2],
                        in0=ssum[:, 2 * h + 1:2 * h + 2],
                        in1=part[:], op=mybir.AluOpType.add)
                    nc.vector.reduce_sum(out=part[:],
                                         in_=xtpT_hs[:, h * NTL:(h + 1) * NTL],
                                         axis=mybir.AxisListType.X)
                    nc.vector.tensor_tensor(
                        out=ssum[:, 2 * h:2 * h + 1], in0=ssum[:, 2 * h:2 * h + 1],
                        in1=part[:], op=mybir.AluOpType.add)
                    nc.scalar.activation(
                        scr512[:, :NTL], xtpT_hs[:, h * NTL:(h + 1) * NTL],
                        mybir.ActivationFunctionType.Square, accum_out=part[:])
                    nc.vector.tensor_tensor(
                        out=ssum[:, 2 * h + 1:2 * h + 2],
                        in0=ssum[:, 2 * h + 1:2 * h + 2],
                        in1=part[:], op=mybir.AluOpType.add)
                nc.sync.dma_start(st_loc[:], ssum[:])
                nc.gpsimd.collective_compute(
                    "AllReduce", mybir.AluOpType.add,
                    ins=[st_loc[:]], outs=[st_full[:]], replica_groups=rg)
                stf = s2m.tile([32, 8], f32, tag="stf")
                nc.sync.dma_start(stf[:], st_full[:])

                NALL = float(NS + NT)
                # per head h: mean = stf[:,2h]/NALL; var = stf[:,2h+1]/NALL - mean^2*ms*(2-ms)
                scale2 = s2m.tile([32, H], f32, tag="scale2")
                bias2 = s2m.tile([32, H], f32, tag="bias2")
                tmp = s2m.tile([32, 4], f32, tag="gtmp")
                for h in range(H):
                    w_c = gn2_hs_sb[:, 3 * h:3 * h + 1]
                    b_c = gn2_hs_sb[:, 3 * h + 1:3 * h + 2]
                    ms_c = gn2_hs_sb[:, 3 * h + 2:3 * h + 3]
                    mean2 = tmp[:, 0:1]
                    nc.vector.tensor_scalar_mul(mean2, stf[:, 2 * h:2 * h + 1],
                                                float(1.0 / NALL))
                    nc.vector.tensor_scalar(
                        out=tmp[:, 1:2], in0=ms_c, scalar1=-1.0, scalar2=2.0,
                        op0=mybir.AluOpType.mult, op1=mybir.AluOpType.add)
                    nc.vector.tensor_tensor(out=tmp[:, 1:2], in0=tmp[:, 1:2],
                                            in1=ms_c, op=mybir.AluOpType.mult)
                    nc.vector.tensor_tensor(out=tmp[:, 2:3], in0=mean2,
                                            in1=mean2, op=mybir.AluOpType.mult)
                    nc.vector.tensor_tensor(out=tmp[:, 2:3], in0=tmp[:, 2:3],
                                            in1=tmp[:, 1:2],
                                            op=mybir.AluOpType.mult)
                    var2 = tmp[:, 3:4]
                    nc.vector.tensor_scalar_mul(var2, stf[:, 2 * h + 1:2 * h + 2],
                                                float(1.0 / NALL))
                    nc.vector.tensor_tensor(out=var2, in0=var2, in1=tmp[:, 2:3],
                                            op=mybir.AluOpType.subtract)
                    nc.vector.tensor_scalar_add(var2, var2, float(EPS_GN))
                    nc.scalar.sqrt(var2, var2)
                    nc.vector.reciprocal(var2, var2)
                    nc.vector.tensor_tensor(out=scale2[:, h:h + 1], in0=w_c,
                                            in1=var2, op=mybir.AluOpType.mult)
                    nc.vector.tensor_tensor(out=tmp[:, 1:2], in0=mean2, in1=ms_c,
                                            op=mybir.AluOpType.mult)
                    nc.vector.tensor_tensor(out=tmp[:, 1:2], in0=tmp[:, 1:2],
                                            in1=scale2[:, h:h + 1],
                                            op=mybir.AluOpType.mult)
                    nc.vector.tensor_scalar_mul(tmp[:, 1:2], tmp[:, 1:2], -1.0)
                    nc.vector.tensor_tensor(out=bias2[:, h:h + 1], in0=b_c,
                                            in1=tmp[:, 1:2],
                                            op=mybir.AluOpType.add)
                # normalize + relu per head
                xtn_hs = s2m.tile([32, H * NTL], f32, tag="xtn")
                for h in range(H):
                    nc.scalar.activation(xtn_hs[:, h * NTL:(h + 1) * NTL],
                                         xtpT_hs[:, h * NTL:(h + 1) * NTL],
                                         mybir.ActivationFunctionType.Relu,
                                         bias=bias2[:, h:h + 1],
                                         scale=scale2[:, h:h + 1])
                xtn_bf = s2m.tile([32, H * NTL], b16, tag="xtn_bf")
                nc.vector.tensor_copy(xtn_bf[:], xtn_hs[:])
                nc.sync.dma_start(xtT_loc[:], xtn_bf[:])
                nc.gpsimd.collective_compute(
                    "AllGather", mybir.AluOpType.bypass,
                    ins=[xtT_loc[:]], outs=[xtT_stack[:]], replica_groups=rg)

                # ---- adj block (bf16) + minmax + normalize ----
                xtf_bf = s2.tile([32, H * NT], b16, tag="xtf_bf")
                for r in range(M):
                    # stack rows r*32..: [32, 4*256]; scatter head-blocks into
                    # [32, h*2048 + r*256 + t] via strided dest AP
                    nc.sync.dma_start(
                        xtf_bf[:].rearrange("p (h t) -> p h t", h=H)[:, :, r * NTL:(r + 1) * NTL],
                        xtT_stack[r * 32:(r + 1) * 32, :].rearrange(
                            "p (h t) -> p h t", h=H))

                adj_sb = s2.tile([P, 2 * NT], f32, tag="adj")
                mxc = s2m.tile([P, 2], f32, tag="mxc")
                first = True
                for mt in range(2):
                    for nk in range(4):
                        adps = s2ps.tile([P, 512], f32, space="PSUM", tag="ps512")
                        for h in range(H):
                            nc.tensor.matmul(
                                adps[:],
                                xtn_bf[:, h * NTL + mt * P:h * NTL + (mt + 1) * P],
                                xtf_bf[:, h * NT + nk * 512:h * NT + (nk + 1) * 512],
                                start=(h == 0), stop=(h == H - 1))
                        nc.vector.tensor_copy(
                            adj_sb[:, (mt * 4 + nk) * 512:(mt * 4 + nk + 1) * 512],
                            adps[:])
                        tmx = s2m.tile([P, 2], f32, tag="tmx")
                        nc.vector.reduce_max(out=tmx[:, 0:1], in_=adps[:],
                                             axis=mybir.AxisListType.X)
                        nc.vector.tensor_reduce(
                            out=tmx[:, 1:2], in_=adps[:], op=mybir.AluOpType.min,
                            axis=mybir.AxisListType.X)
                        if first:
                            nc.vector.tensor_copy(mxc[:], tmx[:])
                            first = False
                        else:
                            nc.vector.tensor_tensor(
                                out=mxc[:, 0:1], in0=mxc[:, 0:1], in1=tmx[:, 0:1],
                                op=mybir.AluOpType.max)
                            nc.vector.tensor_tensor(
                                out=mxc[:, 1:2], in0=mxc[:, 1:2], in1=tmx[:, 1:2],
                                op=mybir.AluOpType.min)
                nc.vector.tensor_scalar_mul(mxc[:, 1:2], mxc[:, 1:2], -1.0)
                mxt_ps = s2ps.tile([P, 512], f32, space="PSUM", tag="ps512")
                nc.tensor.transpose(mxt_ps[:2, :P], mxc[:], ident[:])
                mxrow = s2m.tile([2, P], f32, tag="mxrow")
                nc.vector.tensor_copy(mxrow[:], mxt_ps[:2, :P])
                mm2 = s2m.tile([2, 1], f32, tag="mm2")
                nc.vector.reduce_max(out=mm2[:], in_=mxrow[:],
                                     axis=mybir.AxisListType.X)
                mm2t_ps = s2ps.tile([P, 512], f32, space="PSUM", tag="ps512")
                nc.tensor.transpose(mm2t_ps[:1, :2], mm2[:], ident[:2, :2])
                mmrow = s2m.tile([1, 8], f32, tag="mmrow")
                nc.vector.memset(mmrow[:], -1e30)
                nc.vector.tensor_copy(mmrow[:, 0:2], mm2t_ps[:1, :2])
                nc.sync.dma_start(mm_loc[:], mmrow[:])
                nc.gpsimd.collective_compute(
                    "AllReduce", mybir.AluOpType.max,
                    ins=[mm_loc[:]], outs=[mm_full[:]], replica_groups=rg)
                mmf = s2m.tile([1, 8], f32, tag="mmf")
                nc.sync.dma_start(mmf[:], mm_full[:])
                sc = s2m.tile([1, 4], f32, tag="scl")
                nc.vector.tensor_tensor(out=sc[:, 0:1], in0=mmf[:, 0:1],
                                        in1=mmf[:, 1:2], op=mybir.AluOpType.add)
                nc.vector.tensor_scalar_add(sc[:, 0:1], sc[:, 0:1], 1e-8)
                nc.vector.reciprocal(sc[:, 1:2], sc[:, 0:1])
                nc.vector.tensor_scalar_mul(sc[:, 2:3], mmf[:, 1:2], -1.0)
                mnrep_ps = s2ps.tile([P, 512], f32, space="PSUM", tag="ps512")
                nc.tensor.matmul(mnrep_ps[:, :2], ones_f32_row[:], sc[:, 1:3],
                                 start=True, stop=True)
                mncol = s2m.tile([P, 2], f32, tag="mncol")
                nc.vector.tensor_copy(mncol[:], mnrep_ps[:, :2])
                for mt in range(2):
                    onorm = s2.tile([P, NT], f32, tag="onorm")
                    nc.vector.tensor_scalar(
                        out=onorm[:], in0=adj_sb[:, mt * NT:(mt + 1) * NT],
                        scalar1=mncol[:, 1:2], scalar2=mncol[:, 0:1],
                        op0=mybir.AluOpType.subtract,
                        op1=mybir.AluOpType.mult)
                    nc.sync.dma_start(adj_out[mt * P:(mt + 1) * P, :], onorm[:])
              if SA < 6:
                # truncated build: write a recognizable dummy output
                z = s2.tile([P, NT], f32, tag="zz")
                nc.vector.memset(z[:], 0.0)
                if SA >= 2:
                    nc.vector.tensor_copy(z[:, 0:NSL], hT_local[:])
                if SA >= 5:
                    nc.vector.tensor_copy(z[:32, NSL:NSL + NTL], xtpT_hs[:, 0:NTL])
                for mt in range(2):
                    nc.sync.dma_start(adj_out[mt * P:(mt + 1) * P, :], z[:])

    nc.compile()
    return nc


def _get_prog(ntile, debug=False):
    stop = os.environ.get("KB_STOP") or None
    key = (ntile, debug, stop)
    if key not in _prog_cache:
        _prog_cache[key] = _build(ntile, debug, stop)
    return _prog_cache[key]


def kernel(**inputs):
    per_core, ntile = _prep(inputs)
    debug = os.environ.get("KB_DEBUG", "0") == "1"
    nc = _get_prog(ntile, debug)
    trace = os.environ.get("KB_TRACE", "0") == "1"
    res = run_bass_kernel_spmd(nc, per_core, core_ids=list(range(M)), trace=trace)
    if trace:
        kernel.last_result = res
    out = np.concatenate([res.results[c]["adj_out"] for c in range(M)], axis=0)
    if debug:
        kernel.debug_results = res.results
    return out



# revision 17
# speedup vs baseline: 1.4310x; 1.4310x over previous
"""Trainium2 Bass kernel for nn_BiMP (GNN message passing), 8 NeuronCores SPMD.

v1 (bf16 + packed-feature stage2 + overlapped collectives):
  stage 1 (sparse TransformerConv, 4096 nodes / 131072 edges, dst-sharded):
    - P1a: kv = x@[Wk|Wv] in bf16 (x^T + W host-cast to bf16), kv table
      (bf16) AllGathered to every core; AllGather overlaps P1b.
    - P1b: [q*isq | skip] = x@[Wq'|Ws]; qWe from q via We-replica mult+reduce;
      per-core q table [512, 132] bf16 written to DRAM for edge gather.
    - edge phase per 128-dst window (4/core, ntile 128-edge tiles each):
      indirect-DMA gather kv rows by src and q rows by dst (both bf16),
      window-batched vector ops for alpha/exp/messages, one-hot S (bf16,
      host-built) scatter-matmuls into PSUM [128, 136]; finalize divides by
      the segment denominator, adds skip, transposes to h^T (bf16).
    - h^T AllGathered in two halves (first half overlaps windows 2-3).
  graph_norm1: per-partition stats on h^T [128, 4096] (redundant per core),
    folded into one Relu activation (scale/bias per partition).
  stage 2 (dense bipartite attention) fully in packed [feat(128), node] layout:
    k2T/q2T/skip2/v2a by single natural matmuls; scores via block-diagonal q2
    ([128, 4*256] rhs, one matmul per 128-source chunk); exp on ACT (bf16);
    numerator+denominator via v2|ones augmented lhsT; per-partition gn2 with
    source-side stats recomputed redundantly from h^T and target-side stats
    from the single xt AllGather; adj = xt@xt.T per 128-row block (bf16),
    min/max AllReduce, normalize.

Self-contained: hardcodes all shapes; compiles on first call (cached per
edge-capacity).
"""
import os
import sys
import types

import numpy as np


def _install_ntff_shim():
    """bass_utils imports antenv.axon_hooks when tracing; provide it."""
    if "antenv.axon_hooks" in sys.modules:
        return
    mod = types.ModuleType("antenv.axon_hooks")

    def set_axon_ntff_profile_hook(h):
        mod._hook = h

    def get_axon_ntff_profile_hook():
        return getattr(mod, "_hook", None)

    mod.set_axon_ntff_profile_hook = set_axon_ntff_profile_hook
    mod.get_axon_ntff_profile_hook = get_axon_ntff_profile_hook
    sys.modules["antenv.axon_hooks"] = mod
    try:
        import antenv
        antenv.axon_hooks = mod
        from trn_agent_boot.trn_boot import _ntff_profile_via_ctypes
        set_axon_ntff_profile_hook(_ntff_profile_via_ctypes("/opt/axon/libaxon_pjrt.so"))
    except Exception:
        pass


_install_ntff_shim()

import ml_dtypes
import concourse.bacc as bacc
import concourse.bass as bass
import concourse.mybir as mybir
import concourse.tile as tile
from concourse.bass_utils import run_bass_kernel_spmd
from concourse.masks import make_identity

dt = mybir.dt
bf16 = ml_dtypes.bfloat16

NS, NT, H, C = 4096, 2048, 4, 32
D = H * C            # 128
E1 = 131072
M = 8                # cores
NSL = NS // M        # 512 source nodes / core
NTL = NT // M        # 256 target rows / core
WIN = 128            # dst nodes per window
NWIN = NSL // WIN    # 4 windows / core
P = 128
ISQ = np.float32(1.0 / np.sqrt(np.float32(C)))
EPS_GN = np.float32(1e-5)
NALL = float(NS + NT)

_prog_cache = {}


# --------------------------------------------------------------------------
# host-side preparation
# --------------------------------------------------------------------------

def _prep(inputs):
    x = np.ascontiguousarray(np.asarray(inputs["x"], np.float32))
    src = np.asarray(inputs["pos_edge_index"][0]).astype(np.int64)
    dst = np.asarray(inputs["pos_edge_index"][1]).astype(np.int64)
    ea = np.asarray(inputs["edge_attr"], np.float32).reshape(-1)
    xt_emb = np.asarray(inputs["target_node_embeddings"], np.float32)

    f32 = lambda k: np.asarray(inputs[k], np.float32)

    We = f32("e1_w").reshape(D)

    # stage-1 weight groups (bf16)
    W2 = np.concatenate([f32("k1_w"), f32("v1_w")], axis=1).astype(bf16)      # [4096,256]
    W1 = np.concatenate([f32("q1_w") * ISQ, f32("skip1_w")], axis=1).astype(bf16)
    B2 = np.concatenate([f32("k1_b"), f32("v1_b")]).reshape(1, 256).astype(bf16)
    B1 = np.concatenate([f32("q1_b") * ISQ, f32("skip1_b")]).reshape(1, 256).astype(bf16)

    # ---- edges: sort by dst, shard by window ----
    order = np.argsort(dst, kind="stable")
    src_s, dst_s, ea_s = src[order], dst[order], ea[order]
    win_id = dst_s // WIN
    counts = np.bincount(win_id, minlength=NS // WIN)
    cap = int(np.ceil(max(int(counts.max()), 128) / 128) * 128)
    ntile = cap // 128
    starts = np.zeros(NS // WIN + 1, np.int64)
    np.cumsum(counts, out=starts[1:])

    xT = x.T  # [feat, node] view

    # stage-2 per-partition columns
    gn1_cols = np.stack([f32("gn1_w"), f32("gn1_b"), f32("gn1_ms")], axis=1)  # [128,3]
    gn2_cols = np.stack([f32("gn2_w"), f32("gn2_b"), f32("gn2_ms")], axis=1)
    # v2 augmented with a ones column per head: [v2_h | 1]
    v2wa = np.zeros((D, 4 * 33), np.float32)
    v2ba = np.zeros((1, 4 * 33), np.float32)
    v2w_np, v2b_np = f32("v2_w"), f32("v2_b")
    for h in range(H):
        v2wa[:, 33 * h:33 * h + 32] = v2w_np[:, 32 * h:32 * (h + 1)]
        v2ba[0, 33 * h:33 * h + 32] = v2b_np[32 * h:32 * (h + 1)]
        v2ba[0, 33 * h + 32] = 1.0
    sel = np.zeros((H, P), np.float32)
    for h in range(H):
        sel[h, 32 * h:32 * (h + 1)] = 1.0

    per_core = []
    for c in range(M):
        eidx = np.zeros((P, NWIN * ntile), np.int32)
        didx = np.zeros((P, NWIN * ntile), np.int32)
        ea_t = np.zeros((P, NWIN * ntile), np.float32)
        S_all = np.zeros((P, NWIN * ntile * P), bf16)
        for w in range(NWIN):
            g = c * NWIN + w
            lo, hi = starts[g], starts[g + 1]
            n = hi - lo
            s_pad = np.zeros(cap, np.int64)
            s_pad[:n] = src_s[lo:hi]
            d_pad = np.full(cap, -1, np.int64)
            d_pad[:n] = dst_s[lo:hi] - g * WIN
            e_pad = np.zeros(cap, np.float32)
            e_pad[:n] = ea_s[lo:hi]
            dq = np.maximum(d_pad, 0) + w * WIN       # local row in q table
            for j in range(ntile):
                sl = slice(j * P, (j + 1) * P)
                col = w * ntile + j
                eidx[:, col] = s_pad[sl]
                didx[:, col] = dq[sl]
                ea_t[:, col] = e_pad[sl]
                dj = d_pad[sl]
                valid = dj >= 0
                Sb = np.zeros((P, P), np.float32)
                Sb[np.arange(P)[valid], dj[valid]] = 1.0
                S_all[:, col * P:(col + 1) * P] = Sb.astype(bf16)
        m = {
            "xT_c": np.ascontiguousarray(xT[:, c * NSL:(c + 1) * NSL]).astype(bf16),
            "W1": W1, "W2": W2, "B1": B1, "B2": B2,
            "S_all": S_all,
            "eidx": eidx, "didx": didx, "ea_t": ea_t,
            "xtT_c": np.ascontiguousarray(xt_emb[c * NTL:(c + 1) * NTL].T).astype(bf16),
            "We_row": We.reshape(1, D),
            "q2w_bf": np.ascontiguousarray(f32("q2_w") * ISQ).astype(bf16),
            "k2w_bf": f32("k2_w").astype(bf16),
            "sk2w_bf": f32("skip2_w").astype(bf16),
            "v2w_aug": v2wa.astype(bf16),
            "v2b_aug": v2ba.astype(bf16),
            "q2b_col": (f32("q2_b") * ISQ).reshape(D, 1),
            "k2b_col": f32("k2_b").reshape(D, 1),
            "sk2b_col": f32("skip2_b").reshape(D, 1),
            "sel_bf": sel.astype(bf16),
            "gn1_cols": gn1_cols,
            "gn2_cols": gn2_cols,
            "ones_bf": np.ones((1, 512), np.float32).astype(bf16),
        }
        per_core.append(m)
    return per_core, ntile


# --------------------------------------------------------------------------
# program builder
# --------------------------------------------------------------------------


def _build(ntile, debug=False, stop=None):
    nc = bacc.Bacc("TRN2", target_bir_lowering=False, debug=False, num_devices=M)
    f32 = dt.float32
    b16 = dt.bfloat16
    SA = {"p1": 1, "edge": 2, "gn1": 3, "proj2": 4, "attn": 5}.get(stop, 6)
    NTW = NWIN * ntile            # edge tiles per core

    # ---- I/O ----
    xT_c = nc.dram_tensor("xT_c", [NS, NSL], b16, kind="ExternalInput")
    W1 = nc.dram_tensor("W1", [NS, 256], b16, kind="ExternalInput")
    W2 = nc.dram_tensor("W2", [NS, 256], b16, kind="ExternalInput")
    B1 = nc.dram_tensor("B1", [1, 256], b16, kind="ExternalInput")
    B2 = nc.dram_tensor("B2", [1, 256], b16, kind="ExternalInput")
    S_all = nc.dram_tensor("S_all", [P, NTW * P], b16, kind="ExternalInput")
    eidx = nc.dram_tensor("eidx", [P, NTW], dt.int32, kind="ExternalInput")
    didx = nc.dram_tensor("didx", [P, NTW], dt.int32, kind="ExternalInput")
    ea_t = nc.dram_tensor("ea_t", [P, NTW], f32, kind="ExternalInput")
    xtT_c = nc.dram_tensor("xtT_c", [D, NTL], b16, kind="ExternalInput")
    We_row = nc.dram_tensor("We_row", [1, D], f32, kind="ExternalInput")
    q2w_bf = nc.dram_tensor("q2w_bf", [D, D], b16, kind="ExternalInput")
    k2w_bf = nc.dram_tensor("k2w_bf", [D, D], b16, kind="ExternalInput")
    sk2w_bf = nc.dram_tensor("sk2w_bf", [D, D], b16, kind="ExternalInput")
    v2w_aug = nc.dram_tensor("v2w_aug", [D, 4 * 33], b16, kind="ExternalInput")
    v2b_aug = nc.dram_tensor("v2b_aug", [1, 4 * 33], b16, kind="ExternalInput")
    q2b_col = nc.dram_tensor("q2b_col", [D, 1], f32, kind="ExternalInput")
    k2b_col = nc.dram_tensor("k2b_col", [D, 1], f32, kind="ExternalInput")
    sk2b_col = nc.dram_tensor("sk2b_col", [D, 1], f32, kind="ExternalInput")
    sel_bf = nc.dram_tensor("sel_bf", [H, P], b16, kind="ExternalInput")
    gn1_cols = nc.dram_tensor("gn1_cols", [D, 3], f32, kind="ExternalInput")
    gn2_cols = nc.dram_tensor("gn2_cols", [D, 3], f32, kind="ExternalInput")
    ones_bf_d = nc.dram_tensor("ones_bf", [1, 512], b16, kind="ExternalInput")

    adj_out = nc.dram_tensor("adj_out", [NTL, NT], f32, kind="ExternalOutput")
    if debug:
        dbg_kv = nc.dram_tensor("dbg_kv", [NSL, 256], f32, kind="ExternalOutput")
        dbg_hT = nc.dram_tensor("dbg_hT", [P, NSL], f32, kind="ExternalOutput")
        dbg_xtp = nc.dram_tensor("dbg_xtp", [P, NTL], f32, kind="ExternalOutput")

    # internal DRAM (collective bounce buffers)
    kv_loc = nc.dram_tensor("kv_loc", [NSL, 256], b16)
    kv_full = nc.dram_tensor("kv_full", [NS, 256], b16, addr_space="Shared")
    q_loc = nc.dram_tensor("q_loc", [NSL, 132], b16)
    hT_loc_a = nc.dram_tensor("hT_loc_a", [P, 256], b16)
    hT_loc_b = nc.dram_tensor("hT_loc_b", [P, 256], b16)
    hT_stack_a = nc.dram_tensor("hT_stack_a", [M * P, 256], b16, addr_space="Shared")
    hT_stack_b = nc.dram_tensor("hT_stack_b", [M * P, 256], b16, addr_space="Shared")
    xtT_loc = nc.dram_tensor("xtT_loc", [P, NTL], b16)
    xtT_stack = nc.dram_tensor("xtT_stack", [M * P, NTL], b16, addr_space="Shared")
    mm_loc = nc.dram_tensor("mm_loc", [1, 8], f32)
    mm_full = nc.dram_tensor("mm_full", [1, 8], f32, addr_space="Shared")

    rg = [list(range(M))]

    with tile.TileContext(nc) as tc:
        with (
            tc.tile_pool(name="persist", bufs=1) as pp,
        ):
            # persistent small tiles
            skip_sb = pp.tile([P, NWIN * D], f32, tag="skip1")
            ea_sb = pp.tile([P, NTW], f32, tag="ea")
            nc.sync.dma_start(ea_sb[:], ea_t[:])
            eidx_sb = pp.tile([P, NTW], dt.int32, tag="eidx")
            nc.sync.dma_start(eidx_sb[:], eidx[:])
            didx_sb = pp.tile([P, NTW], dt.int32, tag="didx")
            nc.sync.dma_start(didx_sb[:], didx[:])
            ones_bf_sb = pp.tile([1, 512], b16, tag="ones_bf")
            nc.sync.dma_start(ones_bf_sb[:], ones_bf_d[:])
            We_sb = pp.tile([1, D], f32, tag="We_row")
            nc.sync.dma_start(We_sb[:], We_row[:])
            gn1_sb = pp.tile([D, 3], f32, tag="gn1")
            nc.sync.dma_start(gn1_sb[:], gn1_cols[:])
            gn2_sb = pp.tile([D, 3], f32, tag="gn2")
            nc.sync.dma_start(gn2_sb[:], gn2_cols[:])
            ident = pp.tile([P, P], f32, tag="ident")
            make_identity(nc, ident)
            ident_bf = pp.tile([P, P], b16, tag="ident_bf")
            nc.vector.tensor_copy(ident_bf[:], ident[:])
            hT_local = pp.tile([P, NSL], b16, tag="hT_local")
            ones_f32_row = pp.tile([1, P], f32, tag="ones_f32r")
            nc.vector.memset(ones_f32_row[:], 1.0)
            We_rep = pp.tile([P, P], f32, tag="We_rep")

            # stage-2 small weights (loaded early; used mid/late)
            xtT_sb = pp.tile([D, NTL], b16, tag="xtT")
            nc.sync.dma_start(xtT_sb[:], xtT_c[:])
            q2w_sb = pp.tile([D, D], b16, tag="q2w")
            nc.sync.dma_start(q2w_sb[:], q2w_bf[:])
            k2w_sb = pp.tile([D, D], b16, tag="k2w")
            nc.sync.dma_start(k2w_sb[:], k2w_bf[:])
            sk2w_sb = pp.tile([D, D], b16, tag="sk2w")
            nc.sync.dma_start(sk2w_sb[:], sk2w_bf[:])
            v2wa_sb = pp.tile([D, 132], b16, tag="v2wa")
            nc.sync.dma_start(v2wa_sb[:], v2w_aug[:])
            v2ba_sb = pp.tile([1, 132], b16, tag="v2ba")
            nc.sync.dma_start(v2ba_sb[:], v2b_aug[:])
            q2b_sb = pp.tile([D, 1], f32, tag="q2b")
            nc.sync.dma_start(q2b_sb[:], q2b_col[:])
            k2b_sb = pp.tile([D, 1], f32, tag="k2b")
            nc.sync.dma_start(k2b_sb[:], k2b_col[:])
            sk2b_sb = pp.tile([D, 1], f32, tag="sk2b")
            nc.sync.dma_start(sk2b_sb[:], sk2b_col[:])
            sel_sb = pp.tile([H, P], b16, tag="sel")
            nc.sync.dma_start(sel_sb[:], sel_bf[:])

            # ============ P1: projections (bf16), kv first ============
            with (
                tc.tile_pool(name="wpool", bufs=1) as wp,
                tc.tile_pool(name="p1ps", bufs=2, space="PSUM") as p1ps,
                tc.tile_pool(name="p1sm", bufs=3) as p1m,
            ):
                X_sb = wp.tile([P, 32 * NSL], b16, tag="X")     # full x^T slice
                W2_ch = [wp.tile([P, 256], b16, tag=f"W2_{kt}", name=f"W2c{kt}")
                         for kt in range(32)]
                W1_ch = [wp.tile([P, 256], b16, tag=f"W1_{kt}", name=f"W1c{kt}")
                         for kt in range(32)]
                B1_sb = wp.tile([1, 256], b16, tag="B1")
                B2_sb = wp.tile([1, 256], b16, tag="B2")
                nc.sync.dma_start(B2_sb[:], B2[:])
                nc.sync.dma_start(B1_sb[:], B1[:])
                for kt in range(32):
                    nc.sync.dma_start(W2_ch[kt][:], W2[kt * P:(kt + 1) * P, :])
                    nc.sync.dma_start(X_sb[:, kt * NSL:(kt + 1) * NSL],
                                      xT_c[kt * P:(kt + 1) * P, :])
                for kt in range(32):
                    nc.sync.dma_start(W1_ch[kt][:], W1[kt * P:(kt + 1) * P, :])

                # We replicated [128,128] f32
                werep_ps = p1ps.tile([P, 256], f32, space="PSUM", tag="ps2")
                nc.tensor.matmul(werep_ps[:, :P], ones_f32_row[:], We_sb[:],
                                 start=True, stop=True)
                nc.vector.tensor_copy(We_rep[:], werep_ps[:, :P])

                # P1a: kv
                for mt in range(NWIN):
                    ps2 = p1ps.tile([P, 256], f32, space="PSUM", tag="ps2")
                    nc.tensor.matmul(ps2[:], ones_bf_sb[:, :P], B2_sb[:],
                                     start=True, stop=False)
                    for kt in range(32):
                        nc.tensor.matmul(
                            ps2[:],
                            X_sb[:, kt * NSL + mt * P:kt * NSL + (mt + 1) * P],
                            W2_ch[kt][:], start=False, stop=(kt == 31))
                    kv_st = p1m.tile([P, 256], b16, tag="kvst")
                    nc.vector.tensor_copy(kv_st[:], ps2[:])
                    nc.sync.dma_start(kv_loc[mt * P:(mt + 1) * P, :], kv_st[:])

                # AllGather kv (overlaps P1b below)
                nc.gpsimd.collective_compute(
                    "AllGather", mybir.AluOpType.bypass,
                    ins=[kv_loc[:]], outs=[kv_full[:]], replica_groups=rg)

                # P1b: q/skip (+ qWe), q table to DRAM
                for mt in range(NWIN):
                    ps1 = p1ps.tile([P, 256], f32, space="PSUM", tag="ps1")
                    nc.tensor.matmul(ps1[:], ones_bf_sb[:, :P], B1_sb[:],
                                     start=True, stop=False)
                    for kt in range(32):
                        nc.tensor.matmul(
                            ps1[:],
                            X_sb[:, kt * NSL + mt * P:kt * NSL + (mt + 1) * P],
                            W1_ch[kt][:], start=False, stop=(kt == 31))
                    qwe_t = p1m.tile([P, P], f32, tag="qwe")
                    nc.vector.tensor_tensor(out=qwe_t[:], in0=ps1[:, 0:P],
                                            in1=We_rep[:], op=mybir.AluOpType.mult)
                    qloc_t = p1m.tile([P, 132], b16, tag="qloc")
                    with nc.allow_low_precision(reason="qWe rowsum to bf16"):
                        nc.vector.reduce_sum(
                            out=qloc_t[:, 128:132],
                            in_=qwe_t[:].rearrange("p (h c) -> p h c", h=H),
                            axis=mybir.AxisListType.X)
                    nc.vector.tensor_copy(qloc_t[:, 0:128], ps1[:, 0:P])
                    nc.sync.dma_start(q_loc[mt * P:(mt + 1) * P, :], qloc_t[:])
                    nc.vector.tensor_copy(skip_sb[:, mt * D:(mt + 1) * D],
                                          ps1[:, 128:256])

            # q2 / skip2 target-side projections (independent of stage 1)
            q2bd = pp.tile([P, H * NTL], b16, tag="q2bd")
            xtpT_skip = pp.tile([P, NTL], f32, tag="xtpT_skip")
            with tc.tile_pool(name="q2ps", bufs=2, space="PSUM") as q2ps:
                qps = q2ps.tile([P, NTL], f32, space="PSUM", tag="q2")
                nc.tensor.matmul(qps[:], q2w_sb[:], xtT_sb[:], start=True, stop=True)
                nc.vector.memset(q2bd[:], 0.0)
                for h in range(H):
                    nc.vector.tensor_scalar(
                        out=q2bd[32 * h:32 * (h + 1), h * NTL:(h + 1) * NTL],
                        in0=qps[32 * h:32 * (h + 1), :],
                        scalar1=q2b_sb[32 * h:32 * (h + 1), 0:1], scalar2=None,
                        op0=mybir.AluOpType.add)
                sps = q2ps.tile([P, NTL], f32, space="PSUM", tag="sk2")
                nc.tensor.matmul(sps[:], sk2w_sb[:], xtT_sb[:], start=True, stop=True)
                nc.vector.tensor_scalar(
                    out=xtpT_skip[:], in0=sps[:], scalar1=sk2b_sb[:, 0:1],
                    scalar2=None, op0=mybir.AluOpType.add)

            if debug:
                dkv = pp.tile([P, 256], f32, tag="dkv")

            # ============ edge phase ============
            if SA >= 2:
                with (
                    tc.tile_pool(name="gat", bufs=1) as gp,
                    tc.tile_pool(name="spool", bufs=2) as spl,
                    tc.tile_pool(name="edgesm", bufs=2) as esm,
                    tc.tile_pool(name="edgeps", bufs=2, space="PSUM") as eps,
                    tc.tile_pool(name="aggps", bufs=2, space="PSUM") as aps,
                ):
                    # prefetch all gathers (gpsimd queue: kvAG then these)
                    gkv_w = []
                    gq_w = []
                    for w in range(NWIN):
                        gkv = gp.tile([P, ntile * 256], b16, tag=f"gkv{w}")
                        nc.gpsimd.indirect_dma_start(
                            out=gkv[:], out_offset=None, in_=kv_full[:],
                            in_offset=bass.IndirectOffsetOnAxis(
                                ap=eidx_sb[:, w * ntile:(w + 1) * ntile], axis=0))
                        gq = gp.tile([P, ntile * 132], b16, tag=f"gq{w}")
                        nc.gpsimd.indirect_dma_start(
                            out=gq[:], out_offset=None, in_=q_loc[:],
                            in_offset=bass.IndirectOffsetOnAxis(
                                ap=didx_sb[:, w * ntile:(w + 1) * ntile], axis=0))
                        gkv_w.append(gkv)
                        gq_w.append(gq)

                    for w in range(NWIN):
                        gkv3 = gkv_w[w][:].rearrange("p (j c) -> p j c", j=ntile)
                        gq3 = gq_w[w][:].rearrange("p (j c) -> p j c", j=ntile)
                        S_sb = spl.tile([P, ntile * P], b16, tag="S")
                        nc.sync.dma_start(
                            S_sb[:], S_all[:, w * ntile * P:(w + 1) * ntile * P])
                        ea3 = ea_sb[:, w * ntile:(w + 1) * ntile].unsqueeze(2)

                        qk = esm.tile([P, ntile * 128], b16, tag="qk")
                        nc.vector.tensor_tensor(
                            out=qk[:], in0=gq3[:, :, 0:128], in1=gkv3[:, :, 0:128],
                            op=mybir.AluOpType.mult)
                        al = esm.tile([P, ntile * H], f32, tag="al")
                        nc.vector.reduce_sum(
                            out=al[:],
                            in_=qk[:].rearrange("p (j h c) -> p (j h) c", h=H, c=C),
                            axis=mybir.AxisListType.X)
                        alw = esm.tile([P, ntile * H], f32, tag="alw")
                        nc.vector.tensor_tensor(
                            out=alw[:], in0=gq3[:, :, 128:132],
                            in1=ea3.to_broadcast([P, ntile, H]),
                            op=mybir.AluOpType.mult)
                        nc.vector.tensor_tensor(
                            out=al[:], in0=al[:], in1=alw[:],
                            op=mybir.AluOpType.add)
                        rhs = esm.tile([P, ntile * 136], b16, tag="rhs")
                        rhs3 = rhs[:].rearrange("p (j c) -> p j c", j=ntile)
                        nc.scalar.activation(
                            rhs3[:, :, 0:H], al[:],
                            mybir.ActivationFunctionType.Exp)
                        nc.vector.tensor_tensor(
                            out=rhs3[:, :, H:2 * H], in0=rhs3[:, :, 0:H],
                            in1=ea3.to_broadcast([P, ntile, H]),
                            op=mybir.AluOpType.mult)
                        for h in range(H):
                            nc.vector.tensor_tensor(
                                out=rhs3[:, :, 8 + C * h:8 + C * (h + 1)],
                                in0=gkv3[:, :, 128 + C * h:128 + C * (h + 1)],
                                in1=rhs3[:, :, h:h + 1].to_broadcast([P, ntile, C]),
                                op=mybir.AluOpType.mult)

                        agg_ps = aps.tile([P, 136], f32, space="PSUM", tag="agg")
                        for j in range(ntile):
                            nc.tensor.matmul(
                                agg_ps[:], S_sb[:, j * P:(j + 1) * P],
                                rhs[:, j * 136:(j + 1) * 136],
                                start=(j == 0), stop=(j == ntile - 1))

                        # finalize window
                        invd = esm.tile([P, H], f32, tag="invd")
                        nc.vector.reciprocal(invd[:], agg_ps[:, 0:H])
                        s2we = esm.tile([P, D], f32, tag="s2we")
                        nc.vector.tensor_tensor(
                            out=s2we[:],
                            in0=agg_ps[:, H:2 * H].unsqueeze(2).to_broadcast([P, H, C]),
                            in1=We_rep[:], op=mybir.AluOpType.mult)
                        hpre = esm.tile([P, D], f32, tag="hpre")
                        nc.vector.tensor_tensor(
                            out=hpre[:], in0=agg_ps[:, 8:136], in1=s2we[:],
                            op=mybir.AluOpType.add)
                        nc.vector.tensor_tensor(
                            out=hpre[:], in0=hpre[:],
                            in1=invd[:].unsqueeze(2).to_broadcast([P, H, C]),
                            op=mybir.AluOpType.mult)
                        hpre_bf = esm.tile([P, D], b16, tag="hpre_bf")
                        nc.vector.tensor_tensor(
                            out=hpre_bf[:], in0=hpre[:],
                            in1=skip_sb[:, w * D:(w + 1) * D],
                            op=mybir.AluOpType.add)
                        tr_ps = eps.tile([P, P], b16, space="PSUM", tag="tr")
                        nc.tensor.transpose(tr_ps[:], hpre_bf[:], ident_bf[:])
                        nc.vector.tensor_copy(hT_local[:, w * P:(w + 1) * P], tr_ps[:])
                        if w == 1:
                            st_a = esm.tile([P, 256], b16, tag="sta")
                            nc.vector.tensor_copy(st_a[:], hT_local[:, 0:256])
                            nc.sync.dma_start(hT_loc_a[:], st_a[:])
                            nc.gpsimd.collective_compute(
                                "AllGather", mybir.AluOpType.bypass,
                                ins=[hT_loc_a[:]], outs=[hT_stack_a[:]],
                                replica_groups=rg)
                        if w == 3:
                            st_b = esm.tile([P, 256], b16, tag="stb")
                            nc.vector.tensor_copy(st_b[:], hT_local[:, 256:512])
                            nc.sync.dma_start(hT_loc_b[:], st_b[:])
                            nc.gpsimd.collective_compute(
                                "AllGather", mybir.AluOpType.bypass,
                                ins=[hT_loc_b[:]], outs=[hT_stack_b[:]],
                                replica_groups=rg)

            if debug and SA >= 2:
                dhT = pp.tile([P, NSL], f32, tag="dhT")
                nc.vector.tensor_copy(dhT[:], hT_local[:])
                nc.sync.dma_start(dbg_hT[:], dhT[:])

            with (
                tc.tile_pool(name="s2sb", bufs=1) as s2,
                tc.tile_pool(name="s2sm", bufs=2) as s2m,
            ):
              if SA >= 3:
                # ---- assemble hT_full (bf16) from the two gathered halves ----
                hT_full = s2.tile([P, NS], b16, tag="hT_full")
                for r in range(M):
                    nc.sync.dma_start(hT_full[:, r * NSL:r * NSL + 256],
                                      hT_stack_a[r * P:(r + 1) * P, :])
                for r in range(M):
                    nc.sync.dma_start(hT_full[:, r * NSL + 256:(r + 1) * NSL],
                                      hT_stack_b[r * P:(r + 1) * P, :])

                # ---- gn1: per-partition stats, fold into one Relu ----
                s1 = s2m.tile([P, 1], f32, tag="s1")
                nc.vector.reduce_sum(out=s1[:], in_=hT_full[:],
                                     axis=mybir.AxisListType.X)
                sqscr = s2.tile([P, NS], b16, tag="sqscr")
                s2sum = s2m.tile([P, 1], f32, tag="s2sum")
                nc.scalar.activation(
                    sqscr[:], hT_full[:], mybir.ActivationFunctionType.Square,
                    accum_out=s2sum[:])
                mean = s2m.tile([P, 1], f32, tag="mean")
                nc.vector.tensor_scalar_mul(mean[:], s1[:], float(1.0 / NS))
                msmean = s2m.tile([P, 1], f32, tag="msmean")
                nc.vector.tensor_tensor(out=msmean[:], in0=mean[:],
                                        in1=gn1_sb[:, 2:3], op=mybir.AluOpType.mult)
                # var = E[x^2] - msmean*(2*mean - msmean)
                tmp = s2m.tile([P, 4], f32, tag="gtmp")
                nc.vector.tensor_scalar_mul(tmp[:, 0:1], mean[:], 2.0)
                nc.vector.tensor_tensor(out=tmp[:, 0:1], in0=tmp[:, 0:1],
                                        in1=msmean[:], op=mybir.AluOpType.subtract)
                nc.vector.tensor_tensor(out=tmp[:, 0:1], in0=tmp[:, 0:1],
                                        in1=msmean[:], op=mybir.AluOpType.mult)
                var = s2m.tile([P, 1], f32, tag="var")
                nc.vector.tensor_scalar_mul(var[:], s2sum[:], float(1.0 / NS))
                nc.vector.tensor_tensor(out=var[:], in0=var[:], in1=tmp[:, 0:1],
                                        op=mybir.AluOpType.subtract)
                nc.vector.tensor_scalar_add(var[:], var[:], float(EPS_GN))
                nc.scalar.sqrt(var[:], var[:])
                rstd = s2m.tile([P, 1], f32, tag="rstd")
                nc.vector.reciprocal(rstd[:], var[:])
                scale1 = s2m.tile([P, 1], f32, tag="scale1")
                nc.vector.tensor_tensor(out=scale1[:], in0=gn1_sb[:, 0:1],
                                        in1=rstd[:], op=mybir.AluOpType.mult)
                bias1 = s2m.tile([P, 1], f32, tag="bias1")
                nc.vector.tensor_tensor(out=bias1[:], in0=scale1[:], in1=msmean[:],
                                        op=mybir.AluOpType.mult)
                nc.vector.tensor_scalar_mul(bias1[:], bias1[:], -1.0)
                nc.vector.tensor_tensor(out=bias1[:], in0=gn1_sb[:, 1:2],
                                        in1=bias1[:], op=mybir.AluOpType.add)
                hTn = s2.tile([P, NS], b16, tag="hTn")
                nc.scalar.activation(hTn[:], hT_full[:],
                                     mybir.ActivationFunctionType.Relu,
                                     bias=bias1[:, 0:1], scale=scale1[:, 0:1])

              if SA >= 4:
               with tc.tile_pool(name="s2psA", bufs=2, space="PSUM") as s2ps:
                # ---- gn2 source-side stats (redundant, from hTn) ----
                srcst = s2m.tile([P, 2], f32, tag="srcst")     # [sum, sumsq]
                hsum = s2m.tile([P, 1], f32, tag="hsum")
                nc.vector.reduce_sum(out=hsum[:], in_=hTn[:],
                                     axis=mybir.AxisListType.X)
                hsum_bf = s2m.tile([P, 1], b16, tag="hsum_bf")
                nc.vector.tensor_copy(hsum_bf[:], hsum[:])
                ssps = s2ps.tile([P, 512], f32, space="PSUM", tag="ps512")
                nc.tensor.matmul(ssps[:, 0:1], sk2w_sb[:], hsum_bf[:],
                                 start=True, stop=True)
                nc.vector.tensor_scalar(
                    out=srcst[:, 0:1], in0=sk2b_sb[:, 0:1], scalar1=float(NS),
                    scalar2=0.0, op0=mybir.AluOpType.mult, op1=mybir.AluOpType.add)
                nc.vector.tensor_tensor(out=srcst[:, 0:1], in0=srcst[:, 0:1],
                                        in1=ssps[:, 0:1], op=mybir.AluOpType.add)
                nc.vector.memset(srcst[:, 1:2], 0.0)
                sq_part = s2m.tile([P, 1], f32, tag="sqpart")
                sqs_scr = s2m.tile([P, NSL], b16, tag="sqs_scr")
                for ch in range(M):
                    skps = s2ps.tile([P, 512], f32, space="PSUM", tag="ps512")
                    nc.tensor.matmul(skps[:], sk2w_sb[:],
                                     hTn[:, ch * NSL:(ch + 1) * NSL],
                                     start=True, stop=True)
                    nc.scalar.activation(
                        sqs_scr[:], skps[:], mybir.ActivationFunctionType.Square,
                        bias=sk2b_sb[:, 0:1], accum_out=sq_part[:])
                    nc.vector.tensor_tensor(out=srcst[:, 1:2], in0=srcst[:, 1:2],
                                            in1=sq_part[:], op=mybir.AluOpType.add)

                # ---- k2T (packed) and v2a ----
                k2T_sb = s2.tile([P, NS], b16, tag="k2T")
                for ch in range(M):
                    kps = s2ps.tile([P, 512], f32, space="PSUM", tag="ps512")
                    nc.tensor.matmul(kps[:], k2w_sb[:],
                                     hTn[:, ch * NSL:(ch + 1) * NSL],
                                     start=True, stop=True)
                    nc.vector.tensor_scalar(
                        out=k2T_sb[:, ch * NSL:(ch + 1) * NSL], in0=kps[:],
                        scalar1=k2b_sb[:, 0:1], scalar2=None,
                        op0=mybir.AluOpType.add)
                v2a_sb = s2.tile([P, 32 * 132], b16, tag="v2a")
                for st in range(32):
                    vps = s2ps.tile([P, 512], f32, space="PSUM", tag="ps512")
                    nc.tensor.matmul(vps[:, 0:132], ones_bf_sb[:, :P], v2ba_sb[:],
                                     start=True, stop=False)
                    nc.tensor.matmul(vps[:, 0:132], hTn[:, st * P:(st + 1) * P],
                                     v2wa_sb[:], start=False, stop=True)
                    nc.vector.tensor_copy(v2a_sb[:, st * 132:(st + 1) * 132],
                                          vps[:, 0:132])

              if SA >= 5:
                # ---- attention: block-diag scores / exp / fused numer+den ----
                xtatt = s2m.tile([P, NTL], f32, tag="xtatt")
                dens = s2m.tile([H, NTL], f32, tag="dens")
                nd_sb = s2m.tile([33, H * NTL], f32, tag="ndsb")
                with (
                    tc.tile_pool(name="scpool", bufs=2, space="PSUM") as scpool,
                    tc.tile_pool(name="ndpool", bufs=1, space="PSUM") as ndpool,
                ):
                    # each head's accumulator in its own PSUM bank
                    nd_all = ndpool.tile([33, H * 512], f32, space="PSUM", tag="nd")
                    for st in range(32):
                        scps = scpool.tile([P, H * NTL], f32, space="PSUM", tag="sc")
                        nc.tensor.matmul(scps[:, 0:512],
                                         k2T_sb[:, st * P:(st + 1) * P],
                                         q2bd[:, 0:512], start=True, stop=True)
                        nc.tensor.matmul(scps[:, 512:1024],
                                         k2T_sb[:, st * P:(st + 1) * P],
                                         q2bd[:, 512:1024], start=True, stop=True)
                        exp_sb = s2m.tile([P, H * NTL], b16, tag="expT")
                        nc.scalar.activation(exp_sb[:], scps[:],
                                             mybir.ActivationFunctionType.Exp)
                        for h in range(H):
                            nc.tensor.matmul(
                                nd_all[:, h * 512:h * 512 + NTL],
                                v2a_sb[:, st * 132 + 33 * h:st * 132 + 33 * (h + 1)],
                                exp_sb[:, h * NTL:(h + 1) * NTL],
                                start=(st == 0), stop=(st == 31))
                    for h in range(H):
                        nc.vector.tensor_copy(nd_sb[:, h * NTL:(h + 1) * NTL],
                                              nd_all[:, h * 512:h * 512 + NTL])
                for h in range(H):
                    nc.sync.dma_start(xtatt[32 * h:32 * (h + 1), :],
                                      nd_sb[0:32, h * NTL:(h + 1) * NTL])
                    nc.sync.dma_start(dens[h:h + 1, :],
                                      nd_sb[32:33, h * NTL:(h + 1) * NTL])
                s2ps_cm = tc.tile_pool(name="s2psB", bufs=2, space="PSUM")
                s2ps = s2ps_cm.__enter__()
                dens_bf = s2m.tile([H, NTL], b16, tag="dens_bf")
                nc.vector.tensor_copy(dens_bf[:], dens[:])
                drep_ps = s2ps.tile([P, 512], f32, space="PSUM", tag="ps512")
                nc.tensor.matmul(drep_ps[:, :NTL], sel_sb[:], dens_bf[:],
                                 start=True, stop=True)
                invd2 = s2m.tile([P, NTL], f32, tag="invd2")
                nc.vector.reciprocal(invd2[:], drep_ps[:, :NTL])
                xtp = s2m.tile([P, NTL], f32, tag="xtp")
                nc.vector.tensor_tensor(out=xtp[:], in0=xtatt[:], in1=invd2[:],
                                        op=mybir.AluOpType.mult)
                nc.vector.tensor_tensor(out=xtp[:], in0=xtp[:], in1=xtpT_skip[:],
                                        op=mybir.AluOpType.add)
                if debug:
                    nc.sync.dma_start(dbg_xtp[:], xtp[:])
                xtp_bf = s2m.tile([P, NTL], b16, tag="xtp_bf")
                nc.vector.tensor_copy(xtp_bf[:], xtp[:])
                nc.sync.dma_start(xtT_loc[:], xtp_bf[:])
                nc.gpsimd.collective_compute(
                    "AllGather", mybir.AluOpType.bypass,
                    ins=[xtT_loc[:]], outs=[xtT_stack[:]], replica_groups=rg)

              if SA >= 6:
                # ---- target-side stats from the gathered buffer + gn2 ----
                xtf_bf = s2.tile([P, NT], b16, tag="xtf_bf")
                for r in range(M):
                    nc.sync.dma_start(xtf_bf[:, r * NTL:(r + 1) * NTL],
                                      xtT_stack[r * P:(r + 1) * P, :])
                t1 = s2m.tile([P, 1], f32, tag="t1")
                nc.vector.reduce_sum(out=t1[:], in_=xtf_bf[:],
                                     axis=mybir.AxisListType.X)
                tsq_scr = s2m.tile([P, NT], b16, tag="tsq_scr")
                t2 = s2m.tile([P, 1], f32, tag="t2")
                nc.scalar.activation(
                    tsq_scr[:], xtf_bf[:], mybir.ActivationFunctionType.Square,
                    accum_out=t2[:])

                mean2 = s2m.tile([P, 1], f32, tag="mean2")
                nc.vector.tensor_tensor(out=mean2[:], in0=srcst[:, 0:1],
                                        in1=t1[:], op=mybir.AluOpType.add)
                nc.vector.tensor_scalar_mul(mean2[:], mean2[:], float(1.0 / NALL))
                msm2 = s2m.tile([P, 1], f32, tag="msm2")
                nc.vector.tensor_tensor(out=msm2[:], in0=mean2[:],
                                        in1=gn2_sb[:, 2:3], op=mybir.AluOpType.mult)
                tmp2 = s2m.tile([P, 2], f32, tag="tmp2")
                nc.vector.tensor_scalar_mul(tmp2[:, 0:1], mean2[:], 2.0)
                nc.vector.tensor_tensor(out=tmp2[:, 0:1], in0=tmp2[:, 0:1],
                                        in1=msm2[:], op=mybir.AluOpType.subtract)
                nc.vector.tensor_tensor(out=tmp2[:, 0:1], in0=tmp2[:, 0:1],
                                        in1=msm2[:], op=mybir.AluOpType.mult)
                var2 = s2m.tile([P, 1], f32, tag="var2")
                nc.vector.tensor_tensor(out=var2[:], in0=srcst[:, 1:2],
                                        in1=t2[:], op=mybir.AluOpType.add)
                nc.vector.tensor_scalar_mul(var2[:], var2[:], float(1.0 / NALL))
                nc.vector.tensor_tensor(out=var2[:], in0=var2[:], in1=tmp2[:, 0:1],
                                        op=mybir.AluOpType.subtract)
                nc.vector.tensor_scalar_add(var2[:], var2[:], float(EPS_GN))
                nc.scalar.sqrt(var2[:], var2[:])
                rstd2 = s2m.tile([P, 1], f32, tag="rstd2")
                nc.vector.reciprocal(rstd2[:], var2[:])
                scale2 = s2m.tile([P, 1], f32, tag="scale2")
                nc.vector.tensor_tensor(out=scale2[:], in0=gn2_sb[:, 0:1],
                                        in1=rstd2[:], op=mybir.AluOpType.mult)
                bias2 = s2m.tile([P, 1], f32, tag="bias2")
                nc.vector.tensor_tensor(out=bias2[:], in0=scale2[:], in1=msm2[:],
                                        op=mybir.AluOpType.mult)
                nc.vector.tensor_scalar_mul(bias2[:], bias2[:], -1.0)
                nc.vector.tensor_tensor(out=bias2[:], in0=gn2_sb[:, 1:2],
                                        in1=bias2[:], op=mybir.AluOpType.add)

                xtn_full = s2.tile([P, NT], b16, tag="xtn_full")
                nc.scalar.activation(xtn_full[:], xtf_bf[:],
                                     mybir.ActivationFunctionType.Relu,
                                     bias=bias2[:, 0:1], scale=scale2[:, 0:1])
                xtn_own = s2m.tile([P, 2 * P], b16, tag="xtn_own")
                nc.scalar.activation(xtn_own[:], xtp[:],
                                     mybir.ActivationFunctionType.Relu,
                                     bias=bias2[:, 0:1], scale=scale2[:, 0:1])

                # ---- adj blocks (bf16) + minmax + normalize ----
                adj_sb = s2.tile([P, 2 * NT], f32, tag="adj")
                mxc = s2m.tile([P, 2], f32, tag="mxc")
                first = True
                for mt in range(2):
                    for nk in range(4):
                        adps = s2ps.tile([P, 512], f32, space="PSUM", tag="ps512")
                        nc.tensor.matmul(
                            adps[:], xtn_own[:, mt * P:(mt + 1) * P],
                            xtn_full[:, nk * 512:(nk + 1) * 512],
                            start=True, stop=True)
                        nc.vector.tensor_copy(
                            adj_sb[:, (mt * 4 + nk) * 512:(mt * 4 + nk + 1) * 512],
                            adps[:])
                        tmx = s2m.tile([P, 2], f32, tag="tmx")
                        nc.vector.reduce_max(out=tmx[:, 0:1], in_=adps[:],
                                             axis=mybir.AxisListType.X)
                        nc.vector.tensor_reduce(
                            out=tmx[:, 1:2], in_=adps[:], op=mybir.AluOpType.min,
                            axis=mybir.AxisListType.X)
                        if first:
                            nc.vector.tensor_copy(mxc[:], tmx[:])
                            first = False
                        else:
                            nc.vector.tensor_tensor(
                                out=mxc[:, 0:1], in0=mxc[:, 0:1], in1=tmx[:, 0:1],
                                op=mybir.AluOpType.max)
                            nc.vector.tensor_tensor(
                                out=mxc[:, 1:2], in0=mxc[:, 1:2], in1=tmx[:, 1:2],
                                op=mybir.AluOpType.min)
                nc.vector.tensor_scalar_mul(mxc[:, 1:2], mxc[:, 1:2], -1.0)
                mxt_ps = s2ps.tile([P, 512], f32, space="PSUM", tag="ps512")
                nc.tensor.transpose(mxt_ps[:2, :P], mxc[:], ident[:])
                mxrow = s2m.tile([2, P], f32, tag="mxrow")
                nc.vector.tensor_copy(mxrow[:], mxt_ps[:2, :P])
                mm2 = s2m.tile([2, 1], f32, tag="mm2")
                nc.vector.reduce_max(out=mm2[:], in_=mxrow[:],
                                     axis=mybir.AxisListType.X)
                mm2t_ps = s2ps.tile([P, 512], f32, space="PSUM", tag="ps512")
                nc.tensor.transpose(mm2t_ps[:1, :2], mm2[:], ident[:2, :2])
                mmrow = s2m.tile([1, 8], f32, tag="mmrow")
                nc.vector.memset(mmrow[:], -1e30)
                nc.vector.tensor_copy(mmrow[:, 0:2], mm2t_ps[:1, :2])
                nc.sync.dma_start(mm_loc[:], mmrow[:])
                nc.gpsimd.collective_compute(
                    "AllReduce", mybir.AluOpType.max,
                    ins=[mm_loc[:]], outs=[mm_full[:]], replica_groups=rg)
                mmf = s2m.tile([1, 8], f32, tag="mmf")
                nc.sync.dma_start(mmf[:], mm_full[:])
                sc = s2m.tile([1, 4], f32, tag="scl")
                nc.vector.tensor_tensor(out=sc[:, 0:1], in0=mmf[:, 0:1],
                                        in1=mmf[:, 1:2], op=mybir.AluOpType.add)
                nc.vector.tensor_scalar_add(sc[:, 0:1], sc[:, 0:1], 1e-8)
                nc.vector.reciprocal(sc[:, 1:2], sc[:, 0:1])
                nc.vector.tensor_scalar_mul(sc[:, 2:3], mmf[:, 1:2], -1.0)
                mnrep_ps = s2ps.tile([P, 512], f32, space="PSUM", tag="ps512")
                nc.tensor.matmul(mnrep_ps[:, :2], ones_f32_row[:], sc[:, 1:3],
                                 start=True, stop=True)
                mncol = s2m.tile([P, 2], f32, tag="mncol")
                nc.vector.tensor_copy(mncol[:], mnrep_ps[:, :2])
                for mt in range(2):
                    onorm = s2.tile([P, NT], f32, tag="onorm")
                    nc.vector.tensor_scalar(
                        out=onorm[:], in0=adj_sb[:, mt * NT:(mt + 1) * NT],
                        scalar1=mncol[:, 1:2], scalar2=mncol[:, 0:1],
                        op0=mybir.AluOpType.subtract,
                        op1=mybir.AluOpType.mult)
                    nc.sync.dma_start(adj_out[mt * P:(mt + 1) * P, :], onorm[:])
                s2ps_cm.__exit__(None, None, None)
              if SA < 6:
                # truncated build: write a recognizable dummy output
                z = s2.tile([P, NT], f32, tag="zz")
                nc.vector.memset(z[:], 0.0)
                if SA >= 2:
                    nc.vector.tensor_copy(z[:, 0:NSL], hT_local[:])
                if SA >= 5:
                    nc.vector.tensor_copy(z[:, NSL:NSL + NTL], xtp[:])
                for mt in range(2):
                    nc.sync.dma_start(adj_out[mt * P:(mt + 1) * P, :], z[:])

    nc.compile()
    return nc


def _get_prog(ntile, debug=False):
    stop = os.environ.get("KB_STOP") or None
    key = (ntile, debug, stop)
    if key not in _prog_cache:
        _prog_cache[key] = _build(ntile, debug, stop)
    return _prog_cache[key]


def kernel(**inputs):
    per_core, ntile = _prep(inputs)
    debug = os.environ.get("KB_DEBUG", "0") == "1"
    nc = _get_prog(ntile, debug)
    trace = os.environ.get("KB_TRACE", "0") == "1"
    res = run_bass_kernel_spmd(nc, per_core, core_ids=list(range(M)), trace=trace)
    if trace:
        kernel.last_result = res
    out = np.concatenate([res.results[c]["adj_out"] for c in range(M)], axis=0)
    if debug:
        kernel.debug_results = res.results
    return out


# revision 19
# speedup vs baseline: 1.6149x; 1.1285x over previous
"""Trainium2 Bass kernel for nn_BiMP (GNN message passing), 8 NeuronCores SPMD.

v1 (bf16 + packed-feature stage2 + overlapped collectives):
  stage 1 (sparse TransformerConv, 4096 nodes / 131072 edges, dst-sharded):
    - P1a: kv = x@[Wk|Wv] in bf16 (x^T + W host-cast to bf16), kv table
      (bf16) AllGathered to every core; AllGather overlaps P1b.
    - P1b: [q*isq | skip] = x@[Wq'|Ws]; qWe from q via We-replica mult+reduce;
      per-core q table [512, 132] bf16 written to DRAM for edge gather.
    - edge phase per 128-dst window (4/core, ntile 128-edge tiles each):
      indirect-DMA gather kv rows by src and q rows by dst (both bf16),
      window-batched vector ops for alpha/exp/messages, one-hot S (bf16,
      host-built) scatter-matmuls into PSUM [128, 136]; finalize divides by
      the segment denominator, adds skip, transposes to h^T (bf16).
    - h^T AllGathered in two halves (first half overlaps windows 2-3).
  graph_norm1: per-partition stats on h^T [128, 4096] (redundant per core),
    folded into one Relu activation (scale/bias per partition).
  stage 2 (dense bipartite attention) fully in packed [feat(128), node] layout:
    k2T/q2T/skip2/v2a by single natural matmuls; scores via block-diagonal q2
    ([128, 4*256] rhs, one matmul per 128-source chunk); exp on ACT (bf16);
    numerator+denominator via v2|ones augmented lhsT; per-partition gn2 with
    source-side stats recomputed redundantly from h^T and target-side stats
    from the single xt AllGather; adj = xt@xt.T per 128-row block (bf16),
    min/max AllReduce, normalize.

Self-contained: hardcodes all shapes; compiles on first call (cached per
edge-capacity).
"""
import os
import sys
import types

import numpy as np


def _install_ntff_shim():
    """bass_utils imports antenv.axon_hooks when tracing; provide it."""
    if "antenv.axon_hooks" in sys.modules:
        return
    mod = types.ModuleType("antenv.axon_hooks")

    def set_axon_ntff_profile_hook(h):
        mod._hook = h

    def get_axon_ntff_profile_hook():
        return getattr(mod, "_hook", None)

    mod.set_axon_ntff_profile_hook = set_axon_ntff_profile_hook
    mod.get_axon_ntff_profile_hook = get_axon_ntff_profile_hook
    sys.modules["antenv.axon_hooks"] = mod
    try:
        import antenv
        antenv.axon_hooks = mod
        from trn_agent_boot.trn_boot import _ntff_profile_via_ctypes
        set_axon_ntff_profile_hook(_ntff_profile_via_ctypes("/opt/axon/libaxon_pjrt.so"))
    except Exception:
        pass


_install_ntff_shim()

import ml_dtypes
import concourse.bacc as bacc
import concourse.bass as bass
import concourse.mybir as mybir
import concourse.tile as tile
from concourse.bass_utils import run_bass_kernel_spmd
from concourse.masks import make_identity

dt = mybir.dt
bf16 = ml_dtypes.bfloat16

NS, NT, H, C = 4096, 2048, 4, 32
D = H * C            # 128
E1 = 131072
M = 8                # cores
NSL = NS // M        # 512 source nodes / core
NTL = NT // M        # 256 target rows / core
WIN = 128            # dst nodes per window
NWIN = NSL // WIN    # 4 windows / core
P = 128
ISQ = np.float32(1.0 / np.sqrt(np.float32(C)))
EPS_GN = np.float32(1e-5)
NALL = float(NS + NT)

_prog_cache = {}


# --------------------------------------------------------------------------
# host-side preparation
# --------------------------------------------------------------------------

def _prep(inputs):
    x = np.ascontiguousarray(np.asarray(inputs["x"], np.float32))
    src = np.asarray(inputs["pos_edge_index"][0]).astype(np.int64)
    dst = np.asarray(inputs["pos_edge_index"][1]).astype(np.int64)
    ea = np.asarray(inputs["edge_attr"], np.float32).reshape(-1)
    xt_emb = np.asarray(inputs["target_node_embeddings"], np.float32)

    f32 = lambda k: np.asarray(inputs[k], np.float32)

    We = f32("e1_w").reshape(D)

    # stage-1 weights merged: [k | v | q*isq | skip] (bf16)
    Wc = np.concatenate([f32("k1_w"), f32("v1_w"), f32("q1_w") * ISQ,
                         f32("skip1_w")], axis=1).astype(bf16)   # [4096,512]
    Bc = np.concatenate([f32("k1_b"), f32("v1_b"), f32("q1_b") * ISQ,
                         f32("skip1_b")]).reshape(1, 512).astype(bf16)

    # ---- edges: sort by dst, shard by window ----
    order = np.argsort(dst, kind="stable")
    src_s, dst_s, ea_s = src[order], dst[order], ea[order]
    win_id = dst_s // WIN
    counts = np.bincount(win_id, minlength=NS // WIN)
    cap = int(np.ceil(max(int(counts.max()), 128) / 128) * 128)
    ntile = cap // 128
    starts = np.zeros(NS // WIN + 1, np.int64)
    np.cumsum(counts, out=starts[1:])

    xT = x.T  # [feat, node] view

    # stage-2 per-partition columns
    gn1_cols = np.stack([f32("gn1_w"), f32("gn1_b"), f32("gn1_ms")], axis=1)  # [128,3]
    gn2_cols = np.stack([f32("gn2_w"), f32("gn2_b"), f32("gn2_ms")], axis=1)
    # v2 augmented with a ones column per head: [v2_h | 1]
    v2wa = np.zeros((D, 4 * 33), np.float32)
    v2ba = np.zeros((1, 4 * 33), np.float32)
    v2w_np, v2b_np = f32("v2_w"), f32("v2_b")
    for h in range(H):
        v2wa[:, 33 * h:33 * h + 32] = v2w_np[:, 32 * h:32 * (h + 1)]
        v2ba[0, 33 * h:33 * h + 32] = v2b_np[32 * h:32 * (h + 1)]
        v2ba[0, 33 * h + 32] = 1.0
    sel = np.zeros((H, P), np.float32)
    for h in range(H):
        sel[h, 32 * h:32 * (h + 1)] = 1.0

    per_core = []
    for c in range(M):
        eidx = np.zeros((P, NWIN * ntile), np.int32)
        didx = np.zeros((P, NWIN * ntile), np.int32)
        ea_t = np.zeros((P, NWIN * ntile), np.float32)
        S_all = np.zeros((P, NWIN * ntile * P), bf16)
        for w in range(NWIN):
            g = c * NWIN + w
            lo, hi = starts[g], starts[g + 1]
            n = hi - lo
            s_pad = np.zeros(cap, np.int64)
            s_pad[:n] = src_s[lo:hi]
            d_pad = np.full(cap, -1, np.int64)
            d_pad[:n] = dst_s[lo:hi] - g * WIN
            e_pad = np.zeros(cap, np.float32)
            e_pad[:n] = ea_s[lo:hi]
            dq = np.maximum(d_pad, 0)                 # window-local q row
            for j in range(ntile):
                sl = slice(j * P, (j + 1) * P)
                col = w * ntile + j
                eidx[:, col] = s_pad[sl]
                didx[:, col] = dq[sl]
                ea_t[:, col] = e_pad[sl]
                dj = d_pad[sl]
                valid = dj >= 0
                Sb = np.zeros((P, P), np.float32)
                Sb[np.arange(P)[valid], dj[valid]] = 1.0
                S_all[:, col * P:(col + 1) * P] = Sb.astype(bf16)
        m = {
            "xT_c": np.ascontiguousarray(xT[:, c * NSL:(c + 1) * NSL]).astype(bf16),
            "Wc": Wc, "Bc": Bc,
            "S_all": S_all,
            "eidx": eidx, "didx": didx, "ea_t": ea_t,
            "xtT_c": np.ascontiguousarray(xt_emb[c * NTL:(c + 1) * NTL].T).astype(bf16),
            "We_row": We.reshape(1, D),
            "q2w_bf": np.ascontiguousarray(f32("q2_w") * ISQ).astype(bf16),
            "k2w_bf": f32("k2_w").astype(bf16),
            "sk2w_bf": f32("skip2_w").astype(bf16),
            "v2w_aug": v2wa.astype(bf16),
            "v2b_aug": v2ba.astype(bf16),
            "q2b_col": (f32("q2_b") * ISQ).reshape(D, 1),
            "k2b_col": f32("k2_b").reshape(D, 1),
            "sk2b_col": f32("skip2_b").reshape(D, 1),
            "sel_bf": sel.astype(bf16),
            "gn1_cols": gn1_cols,
            "gn2_cols": gn2_cols,
            "ones_bf": np.ones((1, 512), np.float32).astype(bf16),
        }
        per_core.append(m)
    return per_core, ntile


# --------------------------------------------------------------------------
# program builder
# --------------------------------------------------------------------------


def _build(ntile, debug=False, stop=None):
    nc = bacc.Bacc("TRN2", target_bir_lowering=False, debug=False, num_devices=M)
    f32 = dt.float32
    b16 = dt.bfloat16
    SA = {"p1": 1, "edge": 2, "gn1": 3, "proj2": 4, "attn": 5}.get(stop, 6)
    NTW = NWIN * ntile            # edge tiles per core

    # ---- I/O ----
    xT_c = nc.dram_tensor("xT_c", [NS, NSL], b16, kind="ExternalInput")
    Wc = nc.dram_tensor("Wc", [NS, 512], b16, kind="ExternalInput")
    Bc = nc.dram_tensor("Bc", [1, 512], b16, kind="ExternalInput")
    S_all = nc.dram_tensor("S_all", [P, NTW * P], b16, kind="ExternalInput")
    eidx = nc.dram_tensor("eidx", [P, NTW], dt.int32, kind="ExternalInput")
    didx = nc.dram_tensor("didx", [P, NTW], dt.int32, kind="ExternalInput")
    ea_t = nc.dram_tensor("ea_t", [P, NTW], f32, kind="ExternalInput")
    xtT_c = nc.dram_tensor("xtT_c", [D, NTL], b16, kind="ExternalInput")
    We_row = nc.dram_tensor("We_row", [1, D], f32, kind="ExternalInput")
    q2w_bf = nc.dram_tensor("q2w_bf", [D, D], b16, kind="ExternalInput")
    k2w_bf = nc.dram_tensor("k2w_bf", [D, D], b16, kind="ExternalInput")
    sk2w_bf = nc.dram_tensor("sk2w_bf", [D, D], b16, kind="ExternalInput")
    v2w_aug = nc.dram_tensor("v2w_aug", [D, 4 * 33], b16, kind="ExternalInput")
    v2b_aug = nc.dram_tensor("v2b_aug", [1, 4 * 33], b16, kind="ExternalInput")
    q2b_col = nc.dram_tensor("q2b_col", [D, 1], f32, kind="ExternalInput")
    k2b_col = nc.dram_tensor("k2b_col", [D, 1], f32, kind="ExternalInput")
    sk2b_col = nc.dram_tensor("sk2b_col", [D, 1], f32, kind="ExternalInput")
    sel_bf = nc.dram_tensor("sel_bf", [H, P], b16, kind="ExternalInput")
    gn1_cols = nc.dram_tensor("gn1_cols", [D, 3], f32, kind="ExternalInput")
    gn2_cols = nc.dram_tensor("gn2_cols", [D, 3], f32, kind="ExternalInput")
    ones_bf_d = nc.dram_tensor("ones_bf", [1, 512], b16, kind="ExternalInput")

    adj_out = nc.dram_tensor("adj_out", [NTL, NT], f32, kind="ExternalOutput")
    if debug:
        dbg_kv = nc.dram_tensor("dbg_kv", [NSL, 256], f32, kind="ExternalOutput")
        dbg_hT = nc.dram_tensor("dbg_hT", [P, NSL], f32, kind="ExternalOutput")
        dbg_xtp = nc.dram_tensor("dbg_xtp", [P, NTL], f32, kind="ExternalOutput")

    # internal DRAM (collective bounce buffers)
    kv_loc = nc.dram_tensor("kv_loc", [NSL, 256], b16)
    kv_full = nc.dram_tensor("kv_full", [NS, 256], b16, addr_space="Shared")
    q_loc_w = [nc.dram_tensor(f"q_loc_{w}", [P, 132], b16) for w in range(NWIN)]
    warm_in = nc.dram_tensor("warm_in", [1, 8], f32)
    warm_out = nc.dram_tensor("warm_out", [1, 8], f32, addr_space="Shared")
    hT_loc_a = nc.dram_tensor("hT_loc_a", [P, 256], b16)
    hT_loc_b = nc.dram_tensor("hT_loc_b", [P, 256], b16)
    hT_stack_a = nc.dram_tensor("hT_stack_a", [M * P, 256], b16, addr_space="Shared")
    hT_stack_b = nc.dram_tensor("hT_stack_b", [M * P, 256], b16, addr_space="Shared")
    xtT_loc = nc.dram_tensor("xtT_loc", [P, NTL], b16)
    xtT_stack = nc.dram_tensor("xtT_stack", [M * P, NTL], b16, addr_space="Shared")
    mm_loc = nc.dram_tensor("mm_loc", [1, 8], f32)
    mm_full = nc.dram_tensor("mm_full", [1, 8], f32, addr_space="Shared")

    rg = [list(range(M))]

    with tile.TileContext(nc) as tc:
        with (
            tc.tile_pool(name="persist", bufs=1) as pp,
        ):
            # persistent small tiles
            skip_sb = pp.tile([P, NWIN * D], f32, tag="skip1")
            ea_sb = pp.tile([P, NTW], f32, tag="ea")
            nc.sync.dma_start(ea_sb[:], ea_t[:])
            eidx_sb = pp.tile([P, NTW], dt.int32, tag="eidx")
            nc.sync.dma_start(eidx_sb[:], eidx[:])
            didx_sb = pp.tile([P, NTW], dt.int32, tag="didx")
            nc.sync.dma_start(didx_sb[:], didx[:])
            ones_bf_sb = pp.tile([1, 512], b16, tag="ones_bf")
            nc.sync.dma_start(ones_bf_sb[:], ones_bf_d[:])
            We_sb = pp.tile([1, D], f32, tag="We_row")
            nc.sync.dma_start(We_sb[:], We_row[:])
            gn1_sb = pp.tile([D, 3], f32, tag="gn1")
            nc.sync.dma_start(gn1_sb[:], gn1_cols[:])
            gn2_sb = pp.tile([D, 3], f32, tag="gn2")
            nc.sync.dma_start(gn2_sb[:], gn2_cols[:])
            ident = pp.tile([P, P], f32, tag="ident")
            make_identity(nc, ident)
            ident_bf = pp.tile([P, P], b16, tag="ident_bf")
            nc.vector.tensor_copy(ident_bf[:], ident[:])
            hT_local = pp.tile([P, NSL], b16, tag="hT_local")
            ones_f32_row = pp.tile([1, P], f32, tag="ones_f32r")
            nc.vector.memset(ones_f32_row[:], 1.0)
            warm_sb = pp.tile([1, 8], f32, tag="warm")
            We_rep = pp.tile([P, P], f32, tag="We_rep")

            # stage-2 small weights (loaded early; used mid/late)
            xtT_sb = pp.tile([D, NTL], b16, tag="xtT")
            nc.sync.dma_start(xtT_sb[:], xtT_c[:])
            q2w_sb = pp.tile([D, D], b16, tag="q2w")
            nc.sync.dma_start(q2w_sb[:], q2w_bf[:])
            k2w_sb = pp.tile([D, D], b16, tag="k2w")
            nc.sync.dma_start(k2w_sb[:], k2w_bf[:])
            sk2w_sb = pp.tile([D, D], b16, tag="sk2w")
            nc.sync.dma_start(sk2w_sb[:], sk2w_bf[:])
            v2wa_sb = pp.tile([D, 132], b16, tag="v2wa")
            nc.sync.dma_start(v2wa_sb[:], v2w_aug[:])
            v2ba_sb = pp.tile([1, 132], b16, tag="v2ba")
            nc.sync.dma_start(v2ba_sb[:], v2b_aug[:])
            q2b_sb = pp.tile([D, 1], f32, tag="q2b")
            nc.sync.dma_start(q2b_sb[:], q2b_col[:])
            k2b_sb = pp.tile([D, 1], f32, tag="k2b")
            nc.sync.dma_start(k2b_sb[:], k2b_col[:])
            sk2b_sb = pp.tile([D, 1], f32, tag="sk2b")
            nc.sync.dma_start(sk2b_sb[:], sk2b_col[:])
            sel_sb = pp.tile([H, P], b16, tag="sel")
            nc.sync.dma_start(sel_sb[:], sel_bf[:])

            # ============ P1: merged projections (bf16) ============
            # warm-up collective: absorbs the collectives init barrier
            nc.vector.memset(warm_sb[:], 1.0)
            nc.sync.dma_start(warm_in[:], warm_sb[:])
            nc.gpsimd.collective_compute(
                "AllReduce", mybir.AluOpType.max,
                ins=[warm_in[:]], outs=[warm_out[:]], replica_groups=rg)
            with (
                tc.tile_pool(name="wpool", bufs=1) as wp,
                tc.tile_pool(name="p1ps", bufs=2, space="PSUM") as p1ps,
                tc.tile_pool(name="p1sm", bufs=3) as p1m,
            ):
                X_sb = wp.tile([P, 32 * NSL], b16, tag="X")     # full x^T slice
                W_ch = [wp.tile([P, 512], b16, tag=f"W_{kt}", name=f"Wc{kt}")
                        for kt in range(32)]
                B_sb = wp.tile([1, 512], b16, tag="Bc")
                nc.sync.dma_start(B_sb[:], Bc[:])
                for kt in range(32):
                    nc.sync.dma_start(W_ch[kt][:], Wc[kt * P:(kt + 1) * P, :])
                    nc.sync.dma_start(X_sb[:, kt * NSL:(kt + 1) * NSL],
                                      xT_c[kt * P:(kt + 1) * P, :])

                # We replicated [128,128] f32
                werep_ps = p1ps.tile([P, 512], f32, space="PSUM", tag="ps")
                nc.tensor.matmul(werep_ps[:, :P], ones_f32_row[:], We_sb[:],
                                 start=True, stop=True)
                nc.vector.tensor_copy(We_rep[:], werep_ps[:, :P])

                for mt in range(NWIN):
                    ps = p1ps.tile([P, 512], f32, space="PSUM", tag="ps")
                    nc.tensor.matmul(ps[:], ones_bf_sb[:, :P], B_sb[:],
                                     start=True, stop=False)
                    for kt in range(32):
                        nc.tensor.matmul(
                            ps[:],
                            X_sb[:, kt * NSL + mt * P:kt * NSL + (mt + 1) * P],
                            W_ch[kt][:], start=False, stop=(kt == 31))
                    kv_st = p1m.tile([P, 256], b16, tag="kvst")
                    nc.vector.tensor_copy(kv_st[:], ps[:, 0:256])
                    nc.sync.dma_start(kv_loc[mt * P:(mt + 1) * P, :], kv_st[:])
                    qwe_t = p1m.tile([P, P], f32, tag="qwe")
                    nc.vector.tensor_tensor(out=qwe_t[:], in0=ps[:, 256:384],
                                            in1=We_rep[:], op=mybir.AluOpType.mult)
                    qloc_t = p1m.tile([P, 132], b16, tag="qloc")
                    with nc.allow_low_precision(reason="qWe rowsum to bf16"):
                        nc.vector.reduce_sum(
                            out=qloc_t[:, 128:132],
                            in_=qwe_t[:].rearrange("p (h c) -> p h c", h=H),
                            axis=mybir.AxisListType.X)
                    nc.vector.tensor_copy(qloc_t[:, 0:128], ps[:, 256:384])
                    nc.sync.dma_start(q_loc_w[mt][:], qloc_t[:])
                    nc.vector.tensor_copy(skip_sb[:, mt * D:(mt + 1) * D],
                                          ps[:, 384:512])

                # AllGather kv (first on the gpsimd queue, before gathers)
                nc.gpsimd.collective_compute(
                    "AllGather", mybir.AluOpType.bypass,
                    ins=[kv_loc[:]], outs=[kv_full[:]], replica_groups=rg)

            # q2 / skip2 target-side projections (independent of stage 1)
            q2bd = pp.tile([P, H * NTL], b16, tag="q2bd")
            xtpT_skip = pp.tile([P, NTL], f32, tag="xtpT_skip")
            with tc.tile_pool(name="q2ps", bufs=2, space="PSUM") as q2ps:
                qps = q2ps.tile([P, NTL], f32, space="PSUM", tag="q2")
                nc.tensor.matmul(qps[:], q2w_sb[:], xtT_sb[:], start=True, stop=True)
                nc.vector.memset(q2bd[:], 0.0)
                for h in range(H):
                    nc.vector.tensor_scalar(
                        out=q2bd[32 * h:32 * (h + 1), h * NTL:(h + 1) * NTL],
                        in0=qps[32 * h:32 * (h + 1), :],
                        scalar1=q2b_sb[32 * h:32 * (h + 1), 0:1], scalar2=None,
                        op0=mybir.AluOpType.add)
                sps = q2ps.tile([P, NTL], f32, space="PSUM", tag="sk2")
                nc.tensor.matmul(sps[:], sk2w_sb[:], xtT_sb[:], start=True, stop=True)
                nc.vector.tensor_scalar(
                    out=xtpT_skip[:], in0=sps[:], scalar1=sk2b_sb[:, 0:1],
                    scalar2=None, op0=mybir.AluOpType.add)

            if debug:
                dkv = pp.tile([P, 256], f32, tag="dkv")

            # ============ edge phase ============
            if SA >= 2:
                with (
                    tc.tile_pool(name="gat", bufs=1) as gp,
                    tc.tile_pool(name="spool", bufs=2) as spl,
                    tc.tile_pool(name="edgesm", bufs=2) as esm,
                    tc.tile_pool(name="edgeps", bufs=2, space="PSUM") as eps,
                    tc.tile_pool(name="aggps", bufs=2, space="PSUM") as aps,
                ):
                    # q gathers first (run during the kv AllGather), kv after
                    gkv_w = []
                    gq_w = []
                    for w in range(NWIN):
                        gq = gp.tile([P, ntile * 132], b16, tag=f"gq{w}",
                                     name=f"gq{w}")
                        nc.gpsimd.indirect_dma_start(
                            out=gq[:], out_offset=None, in_=q_loc_w[w][:],
                            in_offset=bass.IndirectOffsetOnAxis(
                                ap=didx_sb[:, w * ntile:(w + 1) * ntile], axis=0))
                        gq_w.append(gq)
                    for w in range(NWIN):
                        gkv = gp.tile([P, ntile * 256], b16, tag=f"gkv{w}",
                                      name=f"gkv{w}")
                        nc.gpsimd.indirect_dma_start(
                            out=gkv[:], out_offset=None, in_=kv_full[:],
                            in_offset=bass.IndirectOffsetOnAxis(
                                ap=eidx_sb[:, w * ntile:(w + 1) * ntile], axis=0))
                        gkv_w.append(gkv)

                    for w in range(NWIN):
                        gkv3 = gkv_w[w][:].rearrange("p (j c) -> p j c", j=ntile)
                        gq3 = gq_w[w][:].rearrange("p (j c) -> p j c", j=ntile)
                        S_sb = spl.tile([P, ntile * P], b16, tag="S")
                        nc.sync.dma_start(
                            S_sb[:], S_all[:, w * ntile * P:(w + 1) * ntile * P])
                        ea3 = ea_sb[:, w * ntile:(w + 1) * ntile].unsqueeze(2)

                        qk = esm.tile([P, ntile * 128], b16, tag="qk")
                        nc.vector.tensor_tensor(
                            out=qk[:], in0=gq3[:, :, 0:128], in1=gkv3[:, :, 0:128],
                            op=mybir.AluOpType.mult)
                        al = esm.tile([P, ntile * H], f32, tag="al")
                        nc.vector.reduce_sum(
                            out=al[:],
                            in_=qk[:].rearrange("p (j h c) -> p (j h) c", h=H, c=C),
                            axis=mybir.AxisListType.X)
                        alw = esm.tile([P, ntile * H], f32, tag="alw")
                        nc.vector.tensor_tensor(
                            out=alw[:], in0=gq3[:, :, 128:132],
                            in1=ea3.to_broadcast([P, ntile, H]),
                            op=mybir.AluOpType.mult)
                        nc.vector.tensor_tensor(
                            out=al[:], in0=al[:], in1=alw[:],
                            op=mybir.AluOpType.add)
                        rhs = esm.tile([P, ntile * 136], b16, tag="rhs")
                        rhs3 = rhs[:].rearrange("p (j c) -> p j c", j=ntile)
                        nc.scalar.activation(
                            rhs3[:, :, 0:H], al[:],
                            mybir.ActivationFunctionType.Exp)
                        nc.vector.tensor_tensor(
                            out=rhs3[:, :, H:2 * H], in0=rhs3[:, :, 0:H],
                            in1=ea3.to_broadcast([P, ntile, H]),
                            op=mybir.AluOpType.mult)
                        for h in range(H):
                            nc.vector.tensor_tensor(
                                out=rhs3[:, :, 8 + C * h:8 + C * (h + 1)],
                                in0=gkv3[:, :, 128 + C * h:128 + C * (h + 1)],
                                in1=rhs3[:, :, h:h + 1].to_broadcast([P, ntile, C]),
                                op=mybir.AluOpType.mult)

                        agg_ps = aps.tile([P, 136], f32, space="PSUM", tag="agg")
                        for j in range(ntile):
                            nc.tensor.matmul(
                                agg_ps[:], S_sb[:, j * P:(j + 1) * P],
                                rhs[:, j * 136:(j + 1) * 136],
                                start=(j == 0), stop=(j == ntile - 1))

                        # finalize window
                        invd = esm.tile([P, H], f32, tag="invd")
                        nc.vector.reciprocal(invd[:], agg_ps[:, 0:H])
                        s2we = esm.tile([P, D], f32, tag="s2we")
                        nc.vector.tensor_tensor(
                            out=s2we[:],
                            in0=agg_ps[:, H:2 * H].unsqueeze(2).to_broadcast([P, H, C]),
                            in1=We_rep[:], op=mybir.AluOpType.mult)
                        hpre = esm.tile([P, D], f32, tag="hpre")
                        nc.vector.tensor_tensor(
                            out=hpre[:], in0=agg_ps[:, 8:136], in1=s2we[:],
                            op=mybir.AluOpType.add)
                        nc.vector.tensor_tensor(
                            out=hpre[:], in0=hpre[:],
                            in1=invd[:].unsqueeze(2).to_broadcast([P, H, C]),
                            op=mybir.AluOpType.mult)
                        hpre_bf = esm.tile([P, D], b16, tag="hpre_bf")
                        nc.vector.tensor_tensor(
                            out=hpre_bf[:], in0=hpre[:],
                            in1=skip_sb[:, w * D:(w + 1) * D],
                            op=mybir.AluOpType.add)
                        tr_ps = eps.tile([P, P], b16, space="PSUM", tag="tr")
                        nc.tensor.transpose(tr_ps[:], hpre_bf[:], ident_bf[:])
                        nc.vector.tensor_copy(hT_local[:, w * P:(w + 1) * P], tr_ps[:])
                        if w == 1:
                            st_a = esm.tile([P, 256], b16, tag="sta")
                            nc.vector.tensor_copy(st_a[:], hT_local[:, 0:256])
                            nc.sync.dma_start(hT_loc_a[:], st_a[:])
                            nc.gpsimd.collective_compute(
                                "AllGather", mybir.AluOpType.bypass,
                                ins=[hT_loc_a[:]], outs=[hT_stack_a[:]],
                                replica_groups=rg)
                        if w == 3:
                            st_b = esm.tile([P, 256], b16, tag="stb")
                            nc.vector.tensor_copy(st_b[:], hT_local[:, 256:512])
                            nc.sync.dma_start(hT_loc_b[:], st_b[:])
                            nc.gpsimd.collective_compute(
                                "AllGather", mybir.AluOpType.bypass,
                                ins=[hT_loc_b[:]], outs=[hT_stack_b[:]],
                                replica_groups=rg)

            if debug and SA >= 2:
                dhT = pp.tile([P, NSL], f32, tag="dhT")
                nc.vector.tensor_copy(dhT[:], hT_local[:])
                nc.sync.dma_start(dbg_hT[:], dhT[:])

            with (
                tc.tile_pool(name="s2sb", bufs=1) as s2,
                tc.tile_pool(name="s2sm", bufs=2) as s2m,
            ):
              if SA >= 3:
                # ---- assemble hT_full (bf16) from the two gathered halves ----
                hT_full = s2.tile([P, NS], b16, tag="hT_full")
                for r in range(M):
                    nc.sync.dma_start(hT_full[:, r * NSL:r * NSL + 256],
                                      hT_stack_a[r * P:(r + 1) * P, :])
                for r in range(M):
                    nc.sync.dma_start(hT_full[:, r * NSL + 256:(r + 1) * NSL],
                                      hT_stack_b[r * P:(r + 1) * P, :])

                # ---- gn1: per-partition stats, fold into one Relu ----
                s1 = s2m.tile([P, 1], f32, tag="s1")
                nc.vector.reduce_sum(out=s1[:], in_=hT_full[:],
                                     axis=mybir.AxisListType.X)
                sqscr = s2.tile([P, NS], b16, tag="sqscr")
                s2sum = s2m.tile([P, 1], f32, tag="s2sum")
                nc.scalar.activation(
                    sqscr[:], hT_full[:], mybir.ActivationFunctionType.Square,
                    accum_out=s2sum[:])
                mean = s2m.tile([P, 1], f32, tag="mean")
                nc.vector.tensor_scalar_mul(mean[:], s1[:], float(1.0 / NS))
                msmean = s2m.tile([P, 1], f32, tag="msmean")
                nc.vector.tensor_tensor(out=msmean[:], in0=mean[:],
                                        in1=gn1_sb[:, 2:3], op=mybir.AluOpType.mult)
                # var = E[x^2] - msmean*(2*mean - msmean)
                tmp = s2m.tile([P, 4], f32, tag="gtmp")
                nc.vector.tensor_scalar_mul(tmp[:, 0:1], mean[:], 2.0)
                nc.vector.tensor_tensor(out=tmp[:, 0:1], in0=tmp[:, 0:1],
                                        in1=msmean[:], op=mybir.AluOpType.subtract)
                nc.vector.tensor_tensor(out=tmp[:, 0:1], in0=tmp[:, 0:1],
                                        in1=msmean[:], op=mybir.AluOpType.mult)
                var = s2m.tile([P, 1], f32, tag="var")
                nc.vector.tensor_scalar_mul(var[:], s2sum[:], float(1.0 / NS))
                nc.vector.tensor_tensor(out=var[:], in0=var[:], in1=tmp[:, 0:1],
                                        op=mybir.AluOpType.subtract)
                nc.vector.tensor_scalar_add(var[:], var[:], float(EPS_GN))
                nc.scalar.sqrt(var[:], var[:])
                rstd = s2m.tile([P, 1], f32, tag="rstd")
                nc.vector.reciprocal(rstd[:], var[:])
                scale1 = s2m.tile([P, 1], f32, tag="scale1")
                nc.vector.tensor_tensor(out=scale1[:], in0=gn1_sb[:, 0:1],
                                        in1=rstd[:], op=mybir.AluOpType.mult)
                bias1 = s2m.tile([P, 1], f32, tag="bias1")
                nc.vector.tensor_tensor(out=bias1[:], in0=scale1[:], in1=msmean[:],
                                        op=mybir.AluOpType.mult)
                nc.vector.tensor_scalar_mul(bias1[:], bias1[:], -1.0)
                nc.vector.tensor_tensor(out=bias1[:], in0=gn1_sb[:, 1:2],
                                        in1=bias1[:], op=mybir.AluOpType.add)
                hTn = s2.tile([P, NS], b16, tag="hTn")
                nc.scalar.activation(hTn[:], hT_full[:],
                                     mybir.ActivationFunctionType.Relu,
                                     bias=bias1[:, 0:1], scale=scale1[:, 0:1])

              if SA >= 4:
               with tc.tile_pool(name="s2psA", bufs=2, space="PSUM") as s2ps:
                # ---- gn2 source-side stats (redundant, from hTn) ----
                srcst = s2m.tile([P, 2], f32, tag="srcst")     # [sum, sumsq]
                hsum = s2m.tile([P, 1], f32, tag="hsum")
                nc.vector.reduce_sum(out=hsum[:], in_=hTn[:],
                                     axis=mybir.AxisListType.X)
                hsum_bf = s2m.tile([P, 1], b16, tag="hsum_bf")
                nc.vector.tensor_copy(hsum_bf[:], hsum[:])
                ssps = s2ps.tile([P, 512], f32, space="PSUM", tag="ps512")
                nc.tensor.matmul(ssps[:, 0:1], sk2w_sb[:], hsum_bf[:],
                                 start=True, stop=True)
                nc.vector.tensor_scalar(
                    out=srcst[:, 0:1], in0=sk2b_sb[:, 0:1], scalar1=float(NS),
                    scalar2=0.0, op0=mybir.AluOpType.mult, op1=mybir.AluOpType.add)
                nc.vector.tensor_tensor(out=srcst[:, 0:1], in0=srcst[:, 0:1],
                                        in1=ssps[:, 0:1], op=mybir.AluOpType.add)
                nc.vector.memset(srcst[:, 1:2], 0.0)
                sq_part = s2m.tile([P, 1], f32, tag="sqpart")
                sqs_scr = s2m.tile([P, NSL], b16, tag="sqs_scr")
                for ch in range(M):
                    skps = s2ps.tile([P, 512], f32, space="PSUM", tag="ps512")
                    nc.tensor.matmul(skps[:], sk2w_sb[:],
                                     hTn[:, ch * NSL:(ch + 1) * NSL],
                                     start=True, stop=True)
                    nc.scalar.activation(
                        sqs_scr[:], skps[:], mybir.ActivationFunctionType.Square,
                        bias=sk2b_sb[:, 0:1], accum_out=sq_part[:])
                    nc.vector.tensor_tensor(out=srcst[:, 1:2], in0=srcst[:, 1:2],
                                            in1=sq_part[:], op=mybir.AluOpType.add)

                # ---- k2T (packed) and v2a ----
                k2T_sb = s2.tile([P, NS], b16, tag="k2T")
                for ch in range(M):
                    kps = s2ps.tile([P, 512], f32, space="PSUM", tag="ps512")
                    nc.tensor.matmul(kps[:], k2w_sb[:],
                                     hTn[:, ch * NSL:(ch + 1) * NSL],
                                     start=True, stop=True)
                    nc.vector.tensor_scalar(
                        out=k2T_sb[:, ch * NSL:(ch + 1) * NSL], in0=kps[:],
                        scalar1=k2b_sb[:, 0:1], scalar2=None,
                        op0=mybir.AluOpType.add)
                v2a_sb = s2.tile([P, 32 * 132], b16, tag="v2a")
                for st in range(32):
                    vps = s2ps.tile([P, 512], f32, space="PSUM", tag="ps512")
                    nc.tensor.matmul(vps[:, 0:132], ones_bf_sb[:, :P], v2ba_sb[:],
                                     start=True, stop=False)
                    nc.tensor.matmul(vps[:, 0:132], hTn[:, st * P:(st + 1) * P],
                                     v2wa_sb[:], start=False, stop=True)
                    nc.vector.tensor_copy(v2a_sb[:, st * 132:(st + 1) * 132],
                                          vps[:, 0:132])

              if SA >= 5:
                # ---- attention: block-diag scores / exp / fused numer+den ----
                xtatt = s2m.tile([P, NTL], f32, tag="xtatt")
                dens = s2m.tile([H, NTL], f32, tag="dens")
                nd_sb = s2m.tile([66, 2 * 512], f32, tag="ndsb")
                with (
                    tc.tile_pool(name="scpool", bufs=2, space="PSUM") as scpool,
                    tc.tile_pool(name="ndpool", bufs=1, space="PSUM") as ndpool,
                ):
                    # head-pair accumulators, one PSUM bank each
                    nd_all = ndpool.tile([66, 2 * 512], f32, space="PSUM", tag="nd")
                    for st in range(32):
                        scps = scpool.tile([P, H * NTL], f32, space="PSUM", tag="sc")
                        nc.tensor.matmul(scps[:, 0:512],
                                         k2T_sb[:, st * P:(st + 1) * P],
                                         q2bd[:, 0:512], start=True, stop=True)
                        nc.tensor.matmul(scps[:, 512:1024],
                                         k2T_sb[:, st * P:(st + 1) * P],
                                         q2bd[:, 512:1024], start=True, stop=True)
                        exp_sb = s2m.tile([P, H * NTL], b16, tag="expT")
                        nc.scalar.activation(exp_sb[:], scps[:],
                                             mybir.ActivationFunctionType.Exp)
                        for hp in range(2):
                            nc.tensor.matmul(
                                nd_all[:, hp * 512:(hp + 1) * 512],
                                v2a_sb[:, st * 132 + 66 * hp:st * 132 + 66 * (hp + 1)],
                                exp_sb[:, 512 * hp:512 * (hp + 1)],
                                start=(st == 0), stop=(st == 31))
                    for hp in range(2):
                        nc.vector.tensor_copy(nd_sb[:, hp * 512:(hp + 1) * 512],
                                              nd_all[:, hp * 512:(hp + 1) * 512])
                for h in range(H):
                    hp, sub = divmod(h, 2)
                    base = hp * 512 + sub * NTL
                    nc.sync.dma_start(
                        xtatt[32 * h:32 * (h + 1), :],
                        nd_sb[33 * sub:33 * sub + 32, base:base + NTL])
                    nc.sync.dma_start(
                        dens[h:h + 1, :],
                        nd_sb[33 * sub + 32:33 * sub + 33, base:base + NTL])
                s2ps_cm = tc.tile_pool(name="s2psB", bufs=2, space="PSUM")
                s2ps = s2ps_cm.__enter__()
                dens_bf = s2m.tile([H, NTL], b16, tag="dens_bf")
                nc.vector.tensor_copy(dens_bf[:], dens[:])
                drep_ps = s2ps.tile([P, 512], f32, space="PSUM", tag="ps512")
                nc.tensor.matmul(drep_ps[:, :NTL], sel_sb[:], dens_bf[:],
                                 start=True, stop=True)
                invd2 = s2m.tile([P, NTL], f32, tag="invd2")
                nc.vector.reciprocal(invd2[:], drep_ps[:, :NTL])
                xtp = s2m.tile([P, NTL], f32, tag="xtp")
                nc.vector.tensor_tensor(out=xtp[:], in0=xtatt[:], in1=invd2[:],
                                        op=mybir.AluOpType.mult)
                nc.vector.tensor_tensor(out=xtp[:], in0=xtp[:], in1=xtpT_skip[:],
                                        op=mybir.AluOpType.add)
                if debug:
                    nc.sync.dma_start(dbg_xtp[:], xtp[:])
                xtp_bf = s2m.tile([P, NTL], b16, tag="xtp_bf")
                nc.vector.tensor_copy(xtp_bf[:], xtp[:])
                nc.sync.dma_start(xtT_loc[:], xtp_bf[:])
                nc.gpsimd.collective_compute(
                    "AllGather", mybir.AluOpType.bypass,
                    ins=[xtT_loc[:]], outs=[xtT_stack[:]], replica_groups=rg)

              if SA >= 6:
                # ---- target-side stats from the gathered buffer + gn2 ----
                xtf_bf = s2.tile([P, NT], b16, tag="xtf_bf")
                for r in range(M):
                    nc.sync.dma_start(xtf_bf[:, r * NTL:(r + 1) * NTL],
                                      xtT_stack[r * P:(r + 1) * P, :])
                t1 = s2m.tile([P, 1], f32, tag="t1")
                nc.vector.reduce_sum(out=t1[:], in_=xtf_bf[:],
                                     axis=mybir.AxisListType.X)
                tsq_scr = s2m.tile([P, NT], b16, tag="tsq_scr")
                t2 = s2m.tile([P, 1], f32, tag="t2")
                nc.scalar.activation(
                    tsq_scr[:], xtf_bf[:], mybir.ActivationFunctionType.Square,
                    accum_out=t2[:])

                mean2 = s2m.tile([P, 1], f32, tag="mean2")
                nc.vector.tensor_tensor(out=mean2[:], in0=srcst[:, 0:1],
                                        in1=t1[:], op=mybir.AluOpType.add)
                nc.vector.tensor_scalar_mul(mean2[:], mean2[:], float(1.0 / NALL))
                msm2 = s2m.tile([P, 1], f32, tag="msm2")
                nc.vector.tensor_tensor(out=msm2[:], in0=mean2[:],
                                        in1=gn2_sb[:, 2:3], op=mybir.AluOpType.mult)
                tmp2 = s2m.tile([P, 2], f32, tag="tmp2")
                nc.vector.tensor_scalar_mul(tmp2[:, 0:1], mean2[:], 2.0)
                nc.vector.tensor_tensor(out=tmp2[:, 0:1], in0=tmp2[:, 0:1],
                                        in1=msm2[:], op=mybir.AluOpType.subtract)
                nc.vector.tensor_tensor(out=tmp2[:, 0:1], in0=tmp2[:, 0:1],
                                        in1=msm2[:], op=mybir.AluOpType.mult)
                var2 = s2m.tile([P, 1], f32, tag="var2")
                nc.vector.tensor_tensor(out=var2[:], in0=srcst[:, 1:2],
                                        in1=t2[:], op=mybir.AluOpType.add)
                nc.vector.tensor_scalar_mul(var2[:], var2[:], float(1.0 / NALL))
                nc.vector.tensor_tensor(out=var2[:], in0=var2[:], in1=tmp2[:, 0:1],
                                        op=mybir.AluOpType.subtract)
                nc.vector.tensor_scalar_add(var2[:], var2[:], float(EPS_GN))
                nc.scalar.sqrt(var2[:], var2[:])
                rstd2 = s2m.tile([P, 1], f32, tag="rstd2")
                nc.vector.reciprocal(rstd2[:], var2[:])
                scale2 = s2m.tile([P, 1], f32, tag="scale2")
                nc.vector.tensor_tensor(out=scale2[:], in0=gn2_sb[:, 0:1],
                                        in1=rstd2[:], op=mybir.AluOpType.mult)
                bias2 = s2m.tile([P, 1], f32, tag="bias2")
                nc.vector.tensor_tensor(out=bias2[:], in0=scale2[:], in1=msm2[:],
                                        op=mybir.AluOpType.mult)
                nc.vector.tensor_scalar_mul(bias2[:], bias2[:], -1.0)
                nc.vector.tensor_tensor(out=bias2[:], in0=gn2_sb[:, 1:2],
                                        in1=bias2[:], op=mybir.AluOpType.add)

                xtn_full = s2.tile([P, NT], b16, tag="xtn_full")
                nc.scalar.activation(xtn_full[:], xtf_bf[:],
                                     mybir.ActivationFunctionType.Relu,
                                     bias=bias2[:, 0:1], scale=scale2[:, 0:1])
                xtn_own = s2m.tile([P, 2 * P], b16, tag="xtn_own")
                nc.scalar.activation(xtn_own[:], xtp[:],
                                     mybir.ActivationFunctionType.Relu,
                                     bias=bias2[:, 0:1], scale=scale2[:, 0:1])

                # ---- adj blocks (bf16) + minmax + normalize ----
                adj_sb = s2.tile([P, 2 * NT], f32, tag="adj")
                mxc = s2m.tile([P, 2], f32, tag="mxc")
                first = True
                for mt in range(2):
                    for nk in range(4):
                        adps = s2ps.tile([P, 512], f32, space="PSUM", tag="ps512")
                        nc.tensor.matmul(
                            adps[:], xtn_own[:, mt * P:(mt + 1) * P],
                            xtn_full[:, nk * 512:(nk + 1) * 512],
                            start=True, stop=True)
                        nc.vector.tensor_copy(
                            adj_sb[:, (mt * 4 + nk) * 512:(mt * 4 + nk + 1) * 512],
                            adps[:])
                        tmx = s2m.tile([P, 2], f32, tag="tmx")
                        nc.vector.reduce_max(out=tmx[:, 0:1], in_=adps[:],
                                             axis=mybir.AxisListType.X)
                        nc.vector.tensor_reduce(
                            out=tmx[:, 1:2], in_=adps[:], op=mybir.AluOpType.min,
                            axis=mybir.AxisListType.X)
                        if first:
                            nc.vector.tensor_copy(mxc[:], tmx[:])
                            first = False
                        else:
                            nc.vector.tensor_tensor(
                                out=mxc[:, 0:1], in0=mxc[:, 0:1], in1=tmx[:, 0:1],
                                op=mybir.AluOpType.max)
                            nc.vector.tensor_tensor(
                                out=mxc[:, 1:2], in0=mxc[:, 1:2], in1=tmx[:, 1:2],
                                op=mybir.AluOpType.min)
                nc.vector.tensor_scalar_mul(mxc[:, 1:2], mxc[:, 1:2], -1.0)
                mxt_ps = s2ps.tile([P, 512], f32, space="PSUM", tag="ps512")
                nc.tensor.transpose(mxt_ps[:2, :P], mxc[:], ident[:])
                mxrow = s2m.tile([2, P], f32, tag="mxrow")
                nc.vector.tensor_copy(mxrow[:], mxt_ps[:2, :P])
                mm2 = s2m.tile([2, 1], f32, tag="mm2")
                nc.vector.reduce_max(out=mm2[:], in_=mxrow[:],
                                     axis=mybir.AxisListType.X)
                mm2t_ps = s2ps.tile([P, 512], f32, space="PSUM", tag="ps512")
                nc.tensor.transpose(mm2t_ps[:1, :2], mm2[:], ident[:2, :2])
                mmrow = s2m.tile([1, 8], f32, tag="mmrow")
                nc.vector.memset(mmrow[:], -1e30)
                nc.vector.tensor_copy(mmrow[:, 0:2], mm2t_ps[:1, :2])
                nc.sync.dma_start(mm_loc[:], mmrow[:])
                nc.gpsimd.collective_compute(
                    "AllReduce", mybir.AluOpType.max,
                    ins=[mm_loc[:]], outs=[mm_full[:]], replica_groups=rg)
                mmf = s2m.tile([1, 8], f32, tag="mmf")
                nc.sync.dma_start(mmf[:], mm_full[:])
                sc = s2m.tile([1, 4], f32, tag="scl")
                nc.vector.tensor_tensor(out=sc[:, 0:1], in0=mmf[:, 0:1],
                                        in1=mmf[:, 1:2], op=mybir.AluOpType.add)
                nc.vector.tensor_scalar_add(sc[:, 0:1], sc[:, 0:1], 1e-8)
                nc.vector.reciprocal(sc[:, 1:2], sc[:, 0:1])
                nc.vector.tensor_scalar_mul(sc[:, 2:3], mmf[:, 1:2], -1.0)
                mnrep_ps = s2ps.tile([P, 512], f32, space="PSUM", tag="ps512")
                nc.tensor.matmul(mnrep_ps[:, :2], ones_f32_row[:], sc[:, 1:3],
                                 start=True, stop=True)
                mncol = s2m.tile([P, 2], f32, tag="mncol")
                nc.vector.tensor_copy(mncol[:], mnrep_ps[:, :2])
                for mt in range(2):
                    onorm = s2.tile([P, NT], f32, tag="onorm")
                    nc.vector.tensor_scalar(
                        out=onorm[:], in0=adj_sb[:, mt * NT:(mt + 1) * NT],
                        scalar1=mncol[:, 1:2], scalar2=mncol[:, 0:1],
                        op0=mybir.AluOpType.subtract,
                        op1=mybir.AluOpType.mult)
                    nc.sync.dma_start(adj_out[mt * P:(mt + 1) * P, :], onorm[:])
                s2ps_cm.__exit__(None, None, None)
              if SA < 6:
                # truncated build: write a recognizable dummy output
                z = s2.tile([P, NT], f32, tag="zz")
                nc.vector.memset(z[:], 0.0)
                if SA >= 2:
                    nc.vector.tensor_copy(z[:, 0:NSL], hT_local[:])
                if SA >= 5:
                    nc.vector.tensor_copy(z[:, NSL:NSL + NTL], xtp[:])
                for mt in range(2):
                    nc.sync.dma_start(adj_out[mt * P:(mt + 1) * P, :], z[:])

    nc.compile()
    return nc


def _get_prog(ntile, debug=False):
    stop = os.environ.get("KB_STOP") or None
    key = (ntile, debug, stop)
    if key not in _prog_cache:
        _prog_cache[key] = _build(ntile, debug, stop)
    return _prog_cache[key]


def kernel(**inputs):
    per_core, ntile = _prep(inputs)
    debug = os.environ.get("KB_DEBUG", "0") == "1"
    nc = _get_prog(ntile, debug)
    trace = os.environ.get("KB_TRACE", "0") == "1"
    res = run_bass_kernel_spmd(nc, per_core, core_ids=list(range(M)), trace=trace)
    if trace:
        kernel.last_result = res
    out = np.concatenate([res.results[c]["adj_out"] for c in range(M)], axis=0)
    if debug:
        kernel.debug_results = res.results
    return out
